# revision 10
# baseline (speedup 1.0000x reference)
"""BiLevelRoutingAttention Trainium2 kernel (8-core SPMD), v2.

Sharding: core r handles batch b = r//4 and windows w in [ (r%4)*8, (r%4)*8+8 ).
Routing (region top-k) is computed on host via linearity of the mean.

Key design points vs v1:
- LIF runs in-place in PSUM with an exact 2^t rescaling of the membrane
  potential: g_t = 2^t * h_t.  Matmuls for step t accumulate 2^(t-1)*x_t@W
  onto the same PSUM bank (start=False), the spike is one is_ge(g, 2^t)
  (threshold scalars 1,2,4,8 are exact), and the hard reset is one
  copy_predicated(psum, spike, zeros) zeroing spiked lanes in place.  No
  h/v state tensors, no add/lt/mult chain.
- q is computed in phase 1 fused with k,v on the same x tile (x loaded once,
  rolling in SBUF); q spikes persist per window for phase 2.
- Attention is token-major: out^T[tok,e] = sq^T @ kv puts 128 tokens on the
  output partitions (PE cost ~ free size only), followed by a cheap f16 PE
  transpose back to feature-major for the projection.
- Spikes are binary so R = k^T v and kv sums are exact integer math in f16
  (counts <= 2048 exact).  attn counts can exceed 2048; f16 rounding there
  only feeds the final linear projection.

QKV matmul precision (KQKV env): "f16x2" (default: x hi+lo f16, w single
f16), "f16x3" (x hi+lo, w hi+lo, 3 cross terms), "float32r" (1 pass).
Q-spike engine (KQE env): "pool" (gpsimd, exact), "dvef16" (DVE, f16 out),
"dvef32r" (DVE, f32r out; q spikes stay 4-byte, attn lhsT f32r).
KV-spike engine (KGE env): "gpsimd" (default; DVE is_ge with fp16 output is
numerically wrong on HW).
"""

import numpy as np
import ml_dtypes
import os as _os

# ---- problem constants (hardcoded per contract) ----
T, B, Lt, Lh, Lw, C = 4, 2, 8, 32, 32, 256
WT, WH, WW = 2, 4, 4
W = WT * WH * WW            # 32 windows
GT, GH, GW = Lt // WT, Lh // WH, Lw // WW
S = GT * GH * GW            # 256 tokens per window
H, D = 8, C // 8            # 8 heads, 32 dim
TOPK = 8
SCALE = float(D) ** -0.5
NCORES = 8
NW = 8                      # windows per core
NTOK = T * S                # 1024 token-instances per window

QKV_MODE = _os.environ.get("KQKV", "f16x2")    # f16x2 | f16x3 | float32r
QE_MODE = _os.environ.get("KQE", "dvef16")     # dvef16 | dvef32r
GE_ENG = _os.environ.get("KGE", "dve")         # kv spike engine: dve (PSUM-capable)

_prog_cache = {}


def _split_sync_waits(nc, mybir, maxw=1):
    """walrus in this container rejects >1 sync wait per instruction; split
    excess waits onto NoOp instructions inserted just before."""
    for bb in nc.main_func.blocks:
        new_list = []
        for ins in bb.instructions:
            si = ins.sync_info
            waits = list(si.on_wait) if si is not None and si.on_wait else []
            if len(waits) > maxw:
                extra = waits[:-maxw]
                keep = waits[-maxw:]
                idx = 0
                while extra:
                    chunk, extra = extra[:maxw], extra[maxw:]
                    nop = mybir.InstNoOp(name=f"{ins.name}-wsplit{idx}", ins=[], outs=[])
                    nop.engine = ins.engine
                    nop.sync_info = mybir.SyncInfo(on_wait=chunk, on_update=[])
                    new_list.append(nop)
                    idx += 1
                ins.sync_info = mybir.SyncInfo(
                    on_wait=keep,
                    on_update=list(si.on_update) if si.on_update else [],
                )
            new_list.append(ins)
        bb.instructions[:] = new_list


def _build_program(routing_idx, sim_mode=False):
    """routing_idx: [B, W, TOPK] int array (host-computed). Returns nc.
    sim_mode: no collective / no If-chain (single-core TimelineSim)."""
    import contextlib
    import concourse.bass as bass
    import concourse.mybir as mybir
    import concourse.tile as tile

    f32 = mybir.dt.float32
    f32r = mybir.dt.float32r
    f16 = mybir.dt.float16
    bf16 = mybir.dt.bfloat16
    ALU = mybir.AluOpType
    ACT = mybir.ActivationFunctionType

    split_x = QKV_MODE in ("f16x2", "f16x3")
    NSPX = 2 if split_x else 1
    NSPW = 2 if QKV_MODE == "f16x3" else 1
    if QKV_MODE == "f16x2":
        PASSES = ((0, 0), (1, 0))
    elif QKV_MODE == "f16x3":
        PASSES = ((0, 0), (0, 1), (1, 0))
    else:
        PASSES = ((0, 0),)
    qkv_dt = f16 if split_x else f32r
    sq_dt = f32r if QE_MODE == "dvef32r" else f16

    nc = bass.Bass(num_devices=NCORES)
    ge_eng = nc.vector if GE_ENG == "dve" else getattr(nc, GE_ENG)
    qe_eng = nc.vector

    # ---- I/O ----
    # x feature-major, per-t pre-scaled by 2^(t-1): [wi, xsplit, kc, c(128), (t,s)]
    x_in = nc.dram_tensor("x_in", [NW, NSPX, 2, 128, NTOK], qkv_dt, kind="ExternalInput")
    wkv_in = nc.dram_tensor("wkv_in", [NSPW, 2, 128, 512], qkv_dt, kind="ExternalInput")
    wq_in = nc.dram_tensor("wq_in", [NSPW, 2, 128, 256], qkv_dt, kind="ExternalInput")
    # bias rows pre-scaled by 2^(t-1), bf16 hi+lo splits: [t, (hi,lo), feat]
    bkv_in = nc.dram_tensor("bkv_in", [T, 2, 512], bf16, kind="ExternalInput")
    bq_in = nc.dram_tensor("bq_in", [T, 2, 256], bf16, kind="ExternalInput")
    wp_in = nc.dram_tensor("wp_in", [2, 128, 256], f16, kind="ExternalInput")
    bp_in = nc.dram_tensor("bp_in", [2, 128, 1], f32, kind="ExternalInput")
    out_d = nc.dram_tensor("out_d", [NW, 2, 128, NTOK], f32, kind="ExternalOutput")
    DBG = _os.environ.get("KDBG") == "1"
    if DBG:
        dbg_skv = nc.dram_tensor("dbg_skv", [NW, 128, 4096], f16, kind="ExternalOutput")
        dbg_sq = nc.dram_tensor("dbg_sq", [NW, 128, 2048],
                                f16 if QE_MODE != "dvef32r" else mybir.dt.float32,
                                kind="ExternalOutput")
        dbg_rloc = nc.dram_tensor("dbg_rloc", [128, 2048], f16, kind="ExternalOutput")
        dbg_kvw = nc.dram_tensor("dbg_kvw", [NW, 128, 256], f16, kind="ExternalOutput")

    with tile.TileContext(nc) as tc:
        with (
            tc.tile_pool(name="const", bufs=1) as constp,
            tc.tile_pool(name="xin", bufs=3) as xin_p,
            tc.tile_pool(name="skv", bufs=2) as skv_p,
            tc.tile_pool(name="persist", bufs=1) as pers_p,
            tc.tile_pool(name="attn", bufs=2) as attn_p,
            tc.tile_pool(name="outs", bufs=2) as out_p,
            tc.tile_pool(name="dram", bufs=1, space="DRAM") as dram_p,
        ):
            # ---- constants / weights ----
            wkv_sb = constp.tile([128, NSPW * 2 * 512], qkv_dt)
            wq_sb = constp.tile([128, NSPW * 2 * 256], qkv_dt)
            for sw in range(NSPW):
                for kc in range(2):
                    nc.sync.dma_start(
                        wkv_sb[:, (sw * 2 + kc) * 512:(sw * 2 + kc + 1) * 512],
                        wkv_in[sw, kc])
                    nc.sync.dma_start(
                        wq_sb[:, (sw * 2 + kc) * 256:(sw * 2 + kc + 1) * 256],
                        wq_in[sw, kc])
            wp_sb = constp.tile([128, 2 * 256], f16)
            for kc in range(2):
                nc.sync.dma_start(wp_sb[:, kc * 256:(kc + 1) * 256], wp_in[kc])
            bp_sb = constp.tile([128, 2], f32)
            for cft in range(2):
                nc.sync.dma_start(bp_sb[:, cft:cft + 1], bp_in[cft])
            bkv_sb = constp.tile([2, T * 512], bf16)     # rows (hi, lo) per t
            bq_sb = constp.tile([2, T * 256], bf16)
            for t in range(T):
                nc.sync.dma_start(bkv_sb[:, t * 512:(t + 1) * 512], bkv_in[t])
                nc.sync.dma_start(bq_sb[:, t * 256:(t + 1) * 256], bq_in[t])
            ones128 = constp.tile([2, 128], bf16)
            nc.vector.memset(ones128[:], 1.0)
            ones256 = constp.tile([2, 256], bf16)
            nc.vector.memset(ones256[:], 1.0)
            zeros = constp.tile([128, 1024], f32)
            nc.vector.memset(zeros[:], 0.0)

            # persistent across phases
            r_loc = pers_p.tile([128, 2048], f16)          # local R, (slab,t,wi,e)
            r_all = pers_p.tile([128, 8192], f16)          # gathered R, (slab,t,j,e)
            kv_w = [pers_p.tile([128, 256], f16, name=f"kvw{i}") for i in range(NW)]
            sq_w = [pers_p.tile([128, 2048], sq_dt, name=f"sqw{i}") for i in range(NW)]

            # ============ phase 1: qkv matmuls + in-place PSUM LIF + R ==========
            ph1 = contextlib.ExitStack()
            ps_kv = ph1.enter_context(tc.tile_pool(name="pskv", bufs=2, space="PSUM"))
            ps_q = ph1.enter_context(tc.tile_pool(name="psq", bufs=2, space="PSUM"))
            ps_r = ph1.enter_context(tc.tile_pool(name="psr", bufs=2, space="PSUM"))

            def xsl(x_sb, sp, kc, lo, hi):
                return x_sb[:, (sp * 2 + kc) * NTOK + lo:(sp * 2 + kc) * NTOK + hi]

            for wpair in range(NW // 2):
                ws = [2 * wpair, 2 * wpair + 1]
                x_t = {}
                pkv = {}
                pq = {}
                skv = {}
                started = {}
                for wi in ws:
                    x_sb = xin_p.tile([128, NSPX * 2 * NTOK], qkv_dt, tag="xsb")
                    for sp in range(NSPX):
                        for kc in range(2):
                            nc.sync.dma_start(
                                x_sb[:, (sp * 2 + kc) * NTOK:(sp * 2 + kc + 1) * NTOK],
                                x_in[wi, sp, kc])
                    x_t[wi] = x_sb
                    pkv[wi] = ps_kv.tile([128, 1024], f32, tag="pkv", name=f"pkv{wi}")
                    pq[wi] = ps_q.tile([128, 512], f32, tag="pq", name=f"pq{wi}")
                    skv[wi] = skv_p.tile([128, 8 * 512], f16, tag="skv", name=f"skv{wi}")
                    # one flag per PSUM zero region (2KB): kv sh0, kv sh1, q
                    started[wi] = [False, False, False]

                def mm(wi, bank, reg, lhsT, rhs):
                    first = not started[wi][bank]
                    started[wi][bank] = True
                    nc.tensor.matmul(reg, lhsT=lhsT, rhs=rhs, start=first,
                                     stop=first, skip_group_check=not first)

                for t in range(T):
                    # ---- kv matmuls (token-major): out [s-half, 512 feat] ----
                    for wi in ws:
                        for sh in range(2):
                            st = t * 2 + sh
                            reg = pkv[wi][:, sh * 512:(sh + 1) * 512]
                            for kc in range(2):
                                for (sx, sw) in PASSES:
                                    mm(wi, sh, reg,
                                       xsl(x_t[wi], sx, kc, st * 128, (st + 1) * 128),
                                       wkv_sb[:, (sw * 2 + kc) * 512:(sw * 2 + kc + 1) * 512])
                            mm(wi, sh, reg, ones128[:],
                               bkv_sb[:, t * 512:(t + 1) * 512])
                    # ---- kv spike + in-place reset ----
                    for wi in ws:
                        sksl = skv[wi][:, (t * 2) * 512:(t * 2 + 2) * 512]
                        ge_eng.tensor_scalar(sksl, pkv[wi][:], float(2 ** t), None,
                                             ALU.is_ge)
                        if t < T - 1:
                            nc.vector.copy_predicated(
                                pkv[wi][:], sksl.bitcast(mybir.dt.uint16), zeros[:])
                    # ---- q matmuls (feature-major): out [feat-half, 256 tok] ----
                    for wi in ws:
                        for ftc in range(2):
                            reg = pq[wi][:, ftc * 256:(ftc + 1) * 256]
                            for kc in range(2):
                                for (sx, sw) in PASSES:
                                    mm(wi, 2, reg,
                                       wq_sb[:, (sw * 2 + kc) * 256 + ftc * 128:
                                             (sw * 2 + kc) * 256 + (ftc + 1) * 128],
                                       xsl(x_t[wi], sx, kc, t * 256, (t + 1) * 256))
                            mm(wi, 2, reg,
                               bq_sb[:, t * 256 + ftc * 128:t * 256 + (ftc + 1) * 128],
                               ones256[:])
                    # ---- q spike + in-place reset ----
                    for wi in ws:
                        sqsl = sq_w[wi][:, t * 512:(t + 1) * 512]
                        qe_eng.tensor_scalar(sqsl, pq[wi][:], float(2 ** t), None,
                                             ALU.is_ge)
                        if t < T - 1:
                            mask_dt = (mybir.dt.uint32 if QE_MODE == "dvef32r"
                                       else mybir.dt.uint16)
                            nc.vector.copy_predicated(
                                pq[wi][:], sqsl.bitcast(mask_dt), zeros[:, :512])

                # ---- R = k^T v per (t, head): [d,e] blocks, col-tiled 4 heads ----
                for wi in ws:
                    for slab in range(2):
                        psr = ps_r.tile([128, 128], f32, tag="psr", name=f"psr{wi}{slab}")
                        for t in range(T):
                            for hl in range(4):
                                h = slab * 4 + hl
                                for sh in range(2):
                                    st = t * 2 + sh
                                    nc.tensor.matmul(
                                        psr[32 * hl:32 * (hl + 1), t * 32:(t + 1) * 32],
                                        lhsT=skv[wi][:, st * 512 + h * 32:
                                                     st * 512 + (h + 1) * 32],
                                        rhs=skv[wi][:, st * 512 + 256 + h * 32:
                                                    st * 512 + 256 + (h + 1) * 32],
                                        start=(sh == 0), stop=(sh == 1),
                                        tile_position=(0, 32 * hl),
                                    )
                        r_view = r_loc[:].rearrange(
                            "p (a t w e) -> p a t w e", a=2, t=4, w=8, e=32)
                        nc.scalar.activation(
                            r_view[:, slab, :, wi, :],
                            psr[:].rearrange("p (t e) -> p t e", t=4, e=32),
                            ACT.Copy, bias=0.0, scale=1.0)
                    if DBG:
                        nc.sync.dma_start(dbg_skv[wi], skv[wi][:])
                        nc.sync.dma_start(dbg_sq[wi], sq_w[wi][:])

            ph1.close()
            # ============ phase 2: exchange R, kv sums, attention, proj =========
            ph2 = contextlib.ExitStack()
            ps_at = ph2.enter_context(tc.tile_pool(name="psat", bufs=2, space="PSUM"))
            ps_pj = ph2.enter_context(tc.tile_pool(name="pspj", bufs=2, space="PSUM"))
            rb_in = dram_p.tile([128, 2048], f16)
            rb_out = dram_p.tile([4, 128, 2048], f16)
            nc.sync.dma_start(rb_in[:], r_loc[:])
            if sim_mode:
                for rk in range(4):
                    nc.sync.dma_start(rb_out[rk], rb_in[:])
            else:
                nc.gpsimd.collective_compute(
                    "AllGather",
                    mybir.AluOpType.bypass,
                    replica_groups=[[0, 1, 2, 3], [4, 5, 6, 7]],
                    ins=[rb_in[:].opt()],
                    outs=[rb_out[:].opt()],
                )
            # r_all free layout: (slab2, t4, j32, e32)
            r_all_v = r_all[:].rearrange("p (a t j e) -> p a t j e", a=2, t=4, j=32, e=32)
            for rk in range(4):
                src = rb_out[rk].rearrange("p (a t w e) -> p a t w e", a=2, t=4, w=8, e=32)
                nc.sync.dma_start(r_all_v[:, :, :, rk * 8:(rk + 1) * 8, :], src)

            # kv sums: routed gather baked per core, guarded by If on core id
            pid = None if sim_mode else nc.partition_id()
            for r in range(NCORES):
                if sim_mode and r != 0:
                    continue
                b_of = r // 4
                wg = r % 4
                with (contextlib.nullcontext() if sim_mode else tc.If(pid == r)):
                    for wl in range(NW):
                        wglob = wg * 8 + wl
                        idxs = [int(j) for j in routing_idx[b_of, wglob]]
                        eng = nc.vector if (wl % 4 == 3) else nc.gpsimd
                        dst = kv_w[wl][:]
                        eng.tensor_copy(dst, r_all_v[:, :, :, idxs[0], :])
                        for j in idxs[1:]:
                            eng.tensor_tensor(
                                dst, dst, r_all_v[:, :, :, j, :], op=ALU.add)

            if DBG:
                nc.sync.dma_start(dbg_rloc[:], r_loc[:])
                for wl in range(NW):
                    nc.sync.dma_start(dbg_kvw[wl], kv_w[wl][:])
            # ---- feature-major attention + projection per window ----
            for wl in range(NW):
                # attn feature-major: [e-feat(128 of slab), (slab, t, tok)]
                attn_sb = attn_p.tile([128, 2 * NTOK], f16, tag="attn")
                attn_v = attn_sb[:].rearrange("p (c t e) -> p c t e",
                                              c=2, t=4, e=256)
                for t in range(T):
                    psa = ps_at.tile([128, 512], f32, tag="psat")
                    for slab in range(2):
                        for hl in range(4):
                            nc.tensor.matmul(
                                psa[32 * hl:32 * (hl + 1),
                                    slab * 256:(slab + 1) * 256],
                                lhsT=kv_w[wl][32 * hl:32 * (hl + 1),
                                              (slab * 4 + t) * 32:
                                              (slab * 4 + t + 1) * 32],
                                rhs=sq_w[wl][32 * hl:32 * (hl + 1),
                                             (t * 2 + slab) * 256:
                                             (t * 2 + slab + 1) * 256],
                                start=True, stop=True,
                                tile_position=(32 * hl, 32 * hl),
                            )
                    nc.scalar.activation(
                        attn_v[:, :, t, :],
                        psa[:].rearrange("p (c e) -> p c e", c=2, e=256),
                        ACT.Copy, bias=0.0, scale=1.0)

                outsb = out_p.tile([128, 2048], f32, tag="outsb")
                for cft in range(2):
                    for nch in range(2):
                        psp = ps_pj.tile([128, 512], f32, tag="pspj")
                        for kc in range(2):
                            nc.tensor.matmul(
                                psp[:],
                                lhsT=wp_sb[:, kc * 256 + cft * 128:
                                           kc * 256 + (cft + 1) * 128],
                                rhs=attn_sb[:, kc * 1024 + nch * 512:
                                            kc * 1024 + (nch + 1) * 512],
                                start=(kc == 0), stop=(kc == 1),
                            )
                        nc.scalar.activation(
                            outsb[:, cft * NTOK + nch * 512:cft * NTOK + (nch + 1) * 512],
                            psp[:], ACT.Identity, bias=bp_sb[:, cft:cft + 1], scale=1.0)
                for cft in range(2):
                    nc.sync.dma_start(out_d[wl, cft], outsb[:, cft * NTOK:(cft + 1) * NTOK])
            ph2.close()

    _split_sync_waits(nc, mybir, maxw=1)
    return nc


def _host_prepost(x, w_qkv, b_qkv):
    """Window partition, routing."""
    xw = x.reshape(T, B, WT, GT, WH, GH, WW, GW, C) \
          .transpose(0, 1, 2, 4, 6, 3, 5, 7, 8).reshape(T, B, W, S, C)
    xbar = xw.mean(axis=(0, 3))                      # [B, W, C]
    q_reg = xbar @ w_qkv[:, :C] + b_qkv[:C]
    k_reg = xbar @ w_qkv[:, C:2 * C] + b_qkv[C:2 * C]
    a_r = np.einsum('bwc,bvc->bwv', q_reg, k_reg)
    routing_idx = np.argsort(-a_r, axis=-1)[:, :, :TOPK]   # [B, W, TOPK]
    return xw, routing_idx


def _hi_lo16(a):
    hi = a.astype(np.float16)
    lo = (a - hi.astype(np.float32)).astype(np.float16)
    return hi, lo


def _hi_lo_bf(a):
    hi = a.astype(ml_dtypes.bfloat16)
    lo = (a - hi.astype(np.float32)).astype(ml_dtypes.bfloat16)
    return hi, lo


def kernel(x, w_qkv, b_qkv, w_proj, b_proj):
    x = np.ascontiguousarray(np.asarray(x, dtype=np.float32))
    w_qkv = np.asarray(w_qkv, dtype=np.float32)
    b_qkv = np.asarray(b_qkv, dtype=np.float32)
    w_proj = np.asarray(w_proj, dtype=np.float32)
    b_proj = np.asarray(b_proj, dtype=np.float32)

    xw, routing_idx = _host_prepost(x, w_qkv, b_qkv)

    key = (routing_idx.tobytes(), QKV_MODE, QE_MODE, GE_ENG)
    if key not in _prog_cache:
        _prog_cache.clear()
        _prog_cache[key] = _build_program(routing_idx)
    nc = _prog_cache[key]

    split_x = QKV_MODE in ("f16x2", "f16x3")
    NSPX = 2 if split_x else 1
    NSPW = 2 if QKV_MODE == "f16x3" else 1
    np_qkv = np.float16 if split_x else np.float32
    tscale = np.array([2.0 ** (t - 1) for t in range(T)], np.float32)

    # weights (shared across cores)
    wkv = w_qkv[:, C:].astype(np.float32)
    wq = w_qkv[:, :C].astype(np.float32)
    if NSPW == 2:
        wkv_hi, wkv_lo = _hi_lo16(wkv)
        wkv_arr = np.stack([wkv_hi, wkv_lo]).reshape(2, 2, 128, 512)
        wq_hi, wq_lo = _hi_lo16(wq)
        wq_arr = np.stack([wq_hi, wq_lo]).reshape(2, 2, 128, 256)
    else:
        wkv_arr = wkv.reshape(1, 2, 128, 512)
        wq_arr = wq.reshape(1, 2, 128, 256)

    bkv_arr = np.empty((T, 2, 512), dtype=ml_dtypes.bfloat16)
    bq_arr = np.empty((T, 2, 256), dtype=ml_dtypes.bfloat16)
    for t in range(T):
        hi, lo = _hi_lo_bf(tscale[t] * b_qkv[C:])
        bkv_arr[t, 0], bkv_arr[t, 1] = hi, lo
        hi, lo = _hi_lo_bf(tscale[t] * b_qkv[:C])
        bq_arr[t, 0], bq_arr[t, 1] = hi, lo

    wp = (SCALE * w_proj).reshape(2, 128, 256).astype(np.float16)
    bp = b_proj.reshape(2, 128, 1).astype(np.float32)

    in_maps = []
    for r in range(NCORES):
        b_of, wg = r // 4, r % 4
        xwc = xw[:, b_of, wg * 8:(wg + 1) * 8]              # [T, 8, S, C]
        xl32 = np.ascontiguousarray(
            xwc.transpose(1, 3, 0, 2))                      # [NW, C, T, S]
        xl32 = xl32 * tscale[None, None, :, None]
        xl32 = xl32.reshape(NW, 2, 128, NTOK)
        if split_x:
            xhi = xl32.astype(np.float16)
            xlo = (xl32 - xhi.astype(np.float32)).astype(np.float16)
            xl = np.stack([xhi, xlo], axis=1)               # [NW, 2, 2, 128, NTOK]
        else:
            xl = xl32.reshape(NW, 1, 2, 128, NTOK)
        in_maps.append({
            "x_in": np.ascontiguousarray(xl.astype(np_qkv)),
            "wkv_in": wkv_arr.astype(np_qkv), "wq_in": wq_arr.astype(np_qkv),
            "bkv_in": bkv_arr, "bq_in": bq_arr,
            "wp_in": wp, "bp_in": bp,
        })

    from concourse.bass_utils import run_bass_kernel_spmd
    res = run_bass_kernel_spmd(nc, in_maps, core_ids=list(range(NCORES)))

    # assemble output
    yw = np.empty((T, B, W, S, C), dtype=np.float32)
    for r in range(NCORES):
        b_of, wg = r // 4, r % 4
        o = res.results[r]["out_d"]                          # [NW, 2, 128, NTOK]
        o = o.reshape(NW, 2, 128, T, S).transpose(0, 3, 4, 1, 2).reshape(NW, T, S, C)
        for wl in range(NW):
            yw[:, b_of, wg * 8 + wl] = o[wl]

    y = yw.reshape(T, B, WT, WH, WW, GT, GH, GW, C) \
          .transpose(0, 1, 2, 5, 3, 6, 4, 7, 8).reshape(T, B, Lt, Lh, Lw, C)
    return y


# revision 19
# speedup vs baseline: 1.4067x; 1.4067x over previous
"""BiLevelRoutingAttention Trainium2 kernel (8-core SPMD), v2.

Sharding: core r handles batch b = r//4 and windows w in [ (r%4)*8, (r%4)*8+8 ).
Routing (region top-k) is computed on host via linearity of the mean.

Key design points vs v1:
- LIF runs in-place in PSUM with an exact 2^t rescaling of the membrane
  potential: g_t = 2^t * h_t.  Matmuls for step t accumulate 2^(t-1)*x_t@W
  onto the same PSUM bank (start=False), the spike is one is_ge(g, 2^t)
  (threshold scalars 1,2,4,8 are exact), and the hard reset is one
  copy_predicated(psum, spike, zeros) zeroing spiked lanes in place.  No
  h/v state tensors, no add/lt/mult chain.
- q is computed in phase 1 fused with k,v on the same x tile (x loaded once,
  rolling in SBUF); q spikes persist per window for phase 2.
- Attention is token-major: out^T[tok,e] = sq^T @ kv puts 128 tokens on the
  output partitions (PE cost ~ free size only), followed by a cheap f16 PE
  transpose back to feature-major for the projection.
- Spikes are binary so R = k^T v and kv sums are exact integer math in f16
  (counts <= 2048 exact).  attn counts can exceed 2048; f16 rounding there
  only feeds the final linear projection.

QKV matmul precision (KQKV env): "f16x2" (default: x hi+lo f16, w single
f16), "f16x3" (x hi+lo, w hi+lo, 3 cross terms), "float32r" (1 pass).
Q-spike engine (KQE env): "pool" (gpsimd, exact), "dvef16" (DVE, f16 out),
"dvef32r" (DVE, f32r out; q spikes stay 4-byte, attn lhsT f32r).
KV-spike engine (KGE env): "gpsimd" (default; DVE is_ge with fp16 output is
numerically wrong on HW).
"""

import numpy as np
import ml_dtypes
import os as _os

# ---- problem constants (hardcoded per contract) ----
T, B, Lt, Lh, Lw, C = 4, 2, 8, 32, 32, 256
WT, WH, WW = 2, 4, 4
W = WT * WH * WW            # 32 windows
GT, GH, GW = Lt // WT, Lh // WH, Lw // WW
S = GT * GH * GW            # 256 tokens per window
H, D = 8, C // 8            # 8 heads, 32 dim
TOPK = 8
SCALE = float(D) ** -0.5
NCORES = 8
NW = 8                      # windows per core
NTOK = T * S                # 1024 token-instances per window

QKV_MODE = _os.environ.get("KQKV", "f16x1")    # f16x1 | f16x2 | f16x3 | float32r
QE_MODE = _os.environ.get("KQE", "actpool")    # actpool | dvef16
GE_ENG = _os.environ.get("KGE", "dve")         # kv spike engine: dve (PSUM-capable)

_prog_cache = {}


def _split_sync_waits(nc, mybir, maxw=1):
    """walrus in this container rejects >1 sync wait per instruction; split
    excess waits onto NoOp instructions inserted just before."""
    for bb in nc.main_func.blocks:
        new_list = []
        for ins in bb.instructions:
            si = ins.sync_info
            waits = list(si.on_wait) if si is not None and si.on_wait else []
            if len(waits) > maxw:
                extra = waits[:-maxw]
                keep = waits[-maxw:]
                idx = 0
                while extra:
                    chunk, extra = extra[:maxw], extra[maxw:]
                    nop = mybir.InstNoOp(name=f"{ins.name}-wsplit{idx}", ins=[], outs=[])
                    nop.engine = ins.engine
                    nop.sync_info = mybir.SyncInfo(on_wait=chunk, on_update=[])
                    new_list.append(nop)
                    idx += 1
                ins.sync_info = mybir.SyncInfo(
                    on_wait=keep,
                    on_update=list(si.on_update) if si.on_update else [],
                )
            new_list.append(ins)
        bb.instructions[:] = new_list


def _build_program(routing_idx, sim_mode=False):
    """routing_idx: [B, W, TOPK] int array (host-computed). Returns nc.
    sim_mode: no collective / no If-chain (single-core TimelineSim)."""
    import contextlib
    import concourse.bass as bass
    import concourse.mybir as mybir
    import concourse.tile as tile

    f32 = mybir.dt.float32
    f32r = mybir.dt.float32r
    f16 = mybir.dt.float16
    bf16 = mybir.dt.bfloat16
    ALU = mybir.AluOpType
    ACT = mybir.ActivationFunctionType
    def reset_mask(ps_ap, g_ap, thr):
        # ps = (g < thr) * g; g is the SBUF snapshot of ps (exact), so only
        # the output touches PSUM (HW: one PSUM operand per DVE op)
        nc.vector.scalar_tensor_tensor(ps_ap, g_ap, thr, g_ap,
                                       ALU.is_lt, ALU.mult)

    split_x = QKV_MODE in ("f16x1", "f16x2", "f16x3")
    NSPX = 2 if QKV_MODE in ("f16x2", "f16x3") else 1
    NSPW = 2 if QKV_MODE == "f16x3" else 1
    if QKV_MODE == "f16x1":
        PASSES = ((0, 0),)
    elif QKV_MODE == "f16x2":
        PASSES = ((0, 0), (1, 0))
    elif QKV_MODE == "f16x3":
        PASSES = ((0, 0), (0, 1), (1, 0))
    else:
        PASSES = ((0, 0),)
    qkv_dt = f16 if split_x else f32r
    sq_dt = f32r if QE_MODE == "dvef32r" else f16

    nc = bass.Bass(num_devices=NCORES)
    ge_eng = nc.vector if GE_ENG == "dve" else getattr(nc, GE_ENG)
    qe_eng = nc.vector

    # ---- I/O ----
    # x feature-major, per-t pre-scaled by 2^(t-1): [wi, xsplit, kc, c(128), (t,s)]
    x_in = nc.dram_tensor("x_in", [NW, NSPX, 2, 128, NTOK], qkv_dt, kind="ExternalInput")
    wkv_in = nc.dram_tensor("wkv_in", [NSPW, 2, 128, 512], qkv_dt, kind="ExternalInput")
    wq_in = nc.dram_tensor("wq_in", [NSPW, 2, 128, 256], qkv_dt, kind="ExternalInput")
    # bias rows pre-scaled by 2^(t-1), bf16 hi+lo splits: [t, (hi,lo), feat]
    bkv_in = nc.dram_tensor("bkv_in", [T, 2, 512], bf16, kind="ExternalInput")
    bq_in = nc.dram_tensor("bq_in", [T, 2, 256], bf16, kind="ExternalInput")
    wp_in = nc.dram_tensor("wp_in", [2, 128, 256], f16, kind="ExternalInput")
    bp_in = nc.dram_tensor("bp_in", [2, 128, 1], f32, kind="ExternalInput")
    out_d = nc.dram_tensor("out_d", [NW, 2, 128, NTOK], f32, kind="ExternalOutput")
    DBG = _os.environ.get("KDBG") == "1"
    ABL = _os.environ.get("KABL", "")
    if DBG:
        dbg_skv = nc.dram_tensor("dbg_skv", [NW, 128, 4096], f16, kind="ExternalOutput")
        dbg_sq = nc.dram_tensor("dbg_sq", [NW, 128, 2048],
                                f16 if QE_MODE != "dvef32r" else mybir.dt.float32,
                                kind="ExternalOutput")
        dbg_rloc = nc.dram_tensor("dbg_rloc", [128, 2048], f16, kind="ExternalOutput")
        dbg_kvw = nc.dram_tensor("dbg_kvw", [NW, 128, 256], f16, kind="ExternalOutput")

    with tile.TileContext(nc) as tc:
        with (
            tc.tile_pool(name="const", bufs=1) as constp,
            tc.tile_pool(name="xin", bufs=3) as xin_p,
            tc.tile_pool(name="skv", bufs=2) as skv_p,
            tc.tile_pool(name="gq", bufs=3) as gq_p,
            tc.tile_pool(name="persist", bufs=1) as pers_p,
            tc.tile_pool(name="attn", bufs=2) as attn_p,
            tc.tile_pool(name="outs", bufs=2) as out_p,
            tc.tile_pool(name="dram", bufs=1, space="DRAM") as dram_p,
        ):
            # ---- constants / weights ----
            wkv_sb = constp.tile([128, NSPW * 2 * 512], qkv_dt)
            wq_sb = constp.tile([128, NSPW * 2 * 256], qkv_dt)
            for sw in range(NSPW):
                for kc in range(2):
                    nc.sync.dma_start(
                        wkv_sb[:, (sw * 2 + kc) * 512:(sw * 2 + kc + 1) * 512],
                        wkv_in[sw, kc])
                    nc.sync.dma_start(
                        wq_sb[:, (sw * 2 + kc) * 256:(sw * 2 + kc + 1) * 256],
                        wq_in[sw, kc])
            wp_sb = constp.tile([128, 2 * 256], f16)
            for kc in range(2):
                nc.sync.dma_start(wp_sb[:, kc * 256:(kc + 1) * 256], wp_in[kc])
            bp_sb = constp.tile([128, 2], f32)
            for cft in range(2):
                nc.sync.dma_start(bp_sb[:, cft:cft + 1], bp_in[cft])
            bkv_sb = constp.tile([2, T * 512], bf16)     # rows (hi, lo) per t
            bq_sb = constp.tile([2, T * 256], bf16)
            for t in range(T):
                nc.sync.dma_start(bkv_sb[:, t * 512:(t + 1) * 512], bkv_in[t])
                nc.sync.dma_start(bq_sb[:, t * 256:(t + 1) * 256], bq_in[t])
            ones128 = constp.tile([2, 128], bf16)
            nc.vector.memset(ones128[:], 1.0)
            ones256 = constp.tile([2, 256], bf16)
            nc.vector.memset(ones256[:], 1.0)
            zeros = constp.tile([128, 1024], f32)
            nc.vector.memset(zeros[:], 0.0)

            # persistent across phases
            r_loc = pers_p.tile([128, 2048], f16)          # local R, (wi,slab,t,e)
            r_rk = [pers_p.tile([128, 2048], f16, name=f"rrk{i}") for i in range(4)]
            kv_w = [pers_p.tile([128, 256], f16, name=f"kvw{i}") for i in range(NW)]
            sq_w = [pers_p.tile([128, 2048], sq_dt, name=f"sqw{i}") for i in range(NW)]

            # ============ phase 1: qkv matmuls + in-place PSUM LIF + R ==========
            ph1 = contextlib.ExitStack()
            ps_kv = ph1.enter_context(tc.tile_pool(name="pskv", bufs=2, space="PSUM"))
            ps_q = ph1.enter_context(tc.tile_pool(name="psq", bufs=2, space="PSUM"))
            ps_r = ph1.enter_context(tc.tile_pool(name="psr", bufs=2, space="PSUM"))

            def xsl(x_sb, sp, kc, lo, hi):
                return x_sb[:, (sp * 2 + kc) * NTOK + lo:(sp * 2 + kc) * NTOK + hi]

            rb_in = [dram_p.tile([128, 1536], f16, name="rbin0"),
                     dram_p.tile([128, 512], f16, name="rbin1")]
            rb_out = [dram_p.tile([4, 128, 1536], f16, name="rbout0"),
                      dram_p.tile([4, 128, 512], f16, name="rbout1")]

            def exchange(half):
                lo, hi = (0, 1536) if half == 0 else (1536, 2048)
                nc.sync.dma_start(rb_in[half][:], r_loc[:, lo:hi])
                if sim_mode:
                    for rk in range(4):
                        nc.sync.dma_start(rb_out[half][rk], rb_in[half][:])
                else:
                    nc.gpsimd.collective_compute(
                        "AllGather",
                        mybir.AluOpType.bypass,
                        replica_groups=[[0, 1, 2, 3], [4, 5, 6, 7]],
                        ins=[rb_in[half][:].opt()],
                        outs=[rb_out[half][:].opt()],
                    )
                for rk in range(4):
                    nc.sync.dma_start(r_rk[rk][:, lo:hi], rb_out[half][rk])

            for wpair in range(NW // 2):
                if wpair == 3:
                    exchange(0)          # windows 0-5 ready; overlaps pair 3
                ws = [2 * wpair, 2 * wpair + 1]
                x_t = {}
                pkv = {}
                pq = {}
                skv = {}
                started = {}
                for wi in ws:
                    x_sb = xin_p.tile([128, NSPX * 2 * NTOK], qkv_dt, tag="xsb")
                    for sp in range(NSPX):
                        for kc in range(2):
                            nc.sync.dma_start(
                                x_sb[:, (sp * 2 + kc) * NTOK:(sp * 2 + kc + 1) * NTOK],
                                x_in[wi, sp, kc])
                    x_t[wi] = x_sb
                    pkv[wi] = ps_kv.tile([128, 1024], f32, tag="pkv", name=f"pkv{wi}")
                    pq[wi] = ps_q.tile([128, 512], f32, tag="pq", name=f"pq{wi}")
                    skv[wi] = skv_p.tile([128, 8 * 512], f16, tag="skv", name=f"skv{wi}")
                    # one flag per PSUM zero region (2KB): kv sh0, kv sh1, q
                    started[wi] = [False, False, False]

                def mm(wi, bank, reg, lhsT, rhs):
                    first = not started[wi][bank]
                    started[wi][bank] = True
                    nc.tensor.matmul(reg, lhsT=lhsT, rhs=rhs, start=first,
                                     stop=first, skip_group_check=not first)

                for t in range(T):
                    # ---- kv matmuls (token-major): out [s-half, 512 feat] ----
                    for wi in ws:
                        for sh in range(2):
                            st = t * 2 + sh
                            reg = pkv[wi][:, sh * 512:(sh + 1) * 512]
                            for kc in range(2):
                                for (sx, sw) in PASSES:
                                    mm(wi, sh, reg,
                                       xsl(x_t[wi], sx, kc, st * 128, (st + 1) * 128),
                                       wkv_sb[:, (sw * 2 + kc) * 512:(sw * 2 + kc + 1) * 512])
                            mm(wi, sh, reg, ones128[:],
                               bkv_sb[:, t * 512:(t + 1) * 512])
                    # ---- kv spike + in-place reset ----
                    for wi in ws:
                        sksl = skv[wi][:, (t * 2) * 512:(t * 2 + 2) * 512]
                        if t < T - 1:
                            gkv = gq_p.tile([128, 1024], f32, tag="gkv",
                                            name=f"gkv{wi}_{t}")
                            nc.scalar.activation(gkv[:], pkv[wi][:], ACT.Copy,
                                                 bias=0.0, scale=1.0)
                            reset_mask(pkv[wi][:], gkv[:], float(2 ** t))
                            nc.gpsimd.tensor_scalar(sksl, gkv[:], float(2 ** t),
                                                    None, ALU.is_ge)
                        else:
                            nc.vector.tensor_scalar(sksl, pkv[wi][:],
                                                    float(2 ** t), None, ALU.is_ge)
                    # ---- q matmuls (feature-major): out [feat-half, 256 tok] ----
                    for wi in ws:
                        for ftc in range(2):
                            reg = pq[wi][:, ftc * 256:(ftc + 1) * 256]
                            for kc in range(2):
                                for (sx, sw) in PASSES:
                                    mm(wi, 2, reg,
                                       wq_sb[:, (sw * 2 + kc) * 256 + ftc * 128:
                                             (sw * 2 + kc) * 256 + (ftc + 1) * 128],
                                       xsl(x_t[wi], sx, kc, t * 256, (t + 1) * 256))
                            mm(wi, 2, reg,
                               bq_sb[:, t * 256 + ftc * 128:t * 256 + (ftc + 1) * 128],
                               ones256[:])
                    # ---- q spike + in-place reset ----
                    for wi in ws:
                        sqsl = sq_w[wi][:, t * 512:(t + 1) * 512]
                        if t < T - 1:
                            gq = gq_p.tile([128, 512], f32, tag="gq",
                                           name=f"gq{wi}_{t}")
                            nc.scalar.activation(gq[:], pq[wi][:], ACT.Copy,
                                                 bias=0.0, scale=1.0)
                            reset_mask(pq[wi][:], gq[:], float(2 ** t))
                            nc.gpsimd.tensor_scalar(sqsl, gq[:], float(2 ** t),
                                                    None, ALU.is_ge)
                        else:
                            nc.vector.tensor_scalar(sqsl, pq[wi][:],
                                                    float(2 ** t), None, ALU.is_ge)

                # ---- R = k^T v per (t, head): [d,e] blocks, col-tiled 4 heads ----
                for wi in ws:
                    for slab in range(2):
                        psr = ps_r.tile([128, 128], f32, tag="psr", name=f"psr{wi}{slab}")
                        for t in range(T):
                            for hl in range(4):
                                h = slab * 4 + hl
                                for sh in range(2):
                                    st = t * 2 + sh
                                    nc.tensor.matmul(
                                        psr[32 * hl:32 * (hl + 1), t * 32:(t + 1) * 32],
                                        lhsT=skv[wi][:, st * 512 + h * 32:
                                                     st * 512 + (h + 1) * 32],
                                        rhs=skv[wi][:, st * 512 + 256 + h * 32:
                                                    st * 512 + 256 + (h + 1) * 32],
                                        start=(sh == 0), stop=(sh == 1),
                                        tile_position=(0, 32 * hl),
                                    )
                        r_view = r_loc[:].rearrange(
                            "p (w a t e) -> p w a t e", w=8, a=2, t=4, e=32)
                        nc.scalar.activation(
                            r_view[:, wi, slab, :, :],
                            psr[:].rearrange("p (t e) -> p t e", t=4, e=32),
                            ACT.Copy, bias=0.0, scale=1.0)
                    if DBG:
                        nc.sync.dma_start(dbg_skv[wi], skv[wi][:])
                        nc.sync.dma_start(dbg_sq[wi], sq_w[wi][:])

            ph1.close()
            # ============ phase 2: finish exchange, kv sums, attention, proj ====
            ph2 = contextlib.ExitStack()
            ps_at = ph2.enter_context(tc.tile_pool(name="psat", bufs=3, space="PSUM"))
            ps_pj = ph2.enter_context(tc.tile_pool(name="pspj", bufs=3, space="PSUM"))
            exchange(1)

            # kv sums: routed gather baked per core, guarded by If on core id
            pid = None if sim_mode else nc.partition_id()
            for r in ([] if ABL == "ph1" else range(NCORES)):
                if sim_mode and r != 0:
                    continue
                b_of = r // 4
                wg = r % 4
                with (contextlib.nullcontext() if sim_mode else tc.If(pid == r)):
                    for wl in range(NW):
                        wglob = wg * 8 + wl
                        idxs = [int(j) for j in routing_idx[b_of, wglob]]
                        idxs.sort(key=lambda j: (j % 8) >= 6)
                        eng = nc.vector if (wl % 2 == 0) else nc.gpsimd

                        def rsrc(j):
                            return r_rk[j // 8][:, (j % 8) * 256:(j % 8 + 1) * 256]
                        dst = kv_w[wl][:]
                        eng.tensor_copy(dst, rsrc(idxs[0]))
                        for j in idxs[1:]:
                            eng.tensor_tensor(dst, dst, rsrc(j), op=ALU.add)

            if DBG:
                nc.sync.dma_start(dbg_rloc[:], r_loc[:])
                for wl in range(NW):
                    nc.sync.dma_start(dbg_kvw[wl], kv_w[wl][:])
            # ---- feature-major attention + projection per window ----
            for wl in ([] if ABL == "ph1" else range(NW)):
                # attn feature-major: [e-feat(128 of slab), (slab, t, tok)]
                attn_sb = attn_p.tile([128, 2 * NTOK], f16, tag="attn")
                attn_v = attn_sb[:].rearrange("p (c t e) -> p c t e",
                                              c=2, t=4, e=256)
                for t in range(T):
                    psa = ps_at.tile([128, 512], f32, tag="psat")
                    for slab in range(2):
                        for hl in range(4):
                            nc.tensor.matmul(
                                psa[32 * hl:32 * (hl + 1),
                                    slab * 256:(slab + 1) * 256],
                                lhsT=kv_w[wl][32 * hl:32 * (hl + 1),
                                              (slab * 4 + t) * 32:
                                              (slab * 4 + t + 1) * 32],
                                rhs=sq_w[wl][32 * hl:32 * (hl + 1),
                                             (t * 2 + slab) * 256:
                                             (t * 2 + slab + 1) * 256],
                                start=True, stop=True,
                                tile_position=(32 * hl, 32 * hl),
                            )
                    nc.vector.tensor_copy(
                        attn_v[:, :, t, :],
                        psa[:].rearrange("p (c e) -> p c e", c=2, e=256))

                outsb = out_p.tile([128, 2048], f32, tag="outsb")
                for cft in range(2):
                    for nch in range(2):
                        psp = ps_pj.tile([128, 512], f32, tag="pspj")
                        for kc in range(2):
                            nc.tensor.matmul(
                                psp[:],
                                lhsT=wp_sb[:, kc * 256 + cft * 128:
                                           kc * 256 + (cft + 1) * 128],
                                rhs=attn_sb[:, kc * 1024 + nch * 512:
                                            kc * 1024 + (nch + 1) * 512],
                                start=(kc == 0), stop=(kc == 1),
                            )
                        nc.scalar.activation(
                            outsb[:, cft * NTOK + nch * 512:cft * NTOK + (nch + 1) * 512],
                            psp[:], ACT.Identity, bias=bp_sb[:, cft:cft + 1], scale=1.0)
                for cft in range(2):
                    nc.sync.dma_start(out_d[wl, cft], outsb[:, cft * NTOK:(cft + 1) * NTOK])
            ph2.close()

    _split_sync_waits(nc, mybir, maxw=1)
    return nc


def _host_prepost(x, w_qkv, b_qkv):
    """Window partition, routing."""
    xw = x.reshape(T, B, WT, GT, WH, GH, WW, GW, C) \
          .transpose(0, 1, 2, 4, 6, 3, 5, 7, 8).reshape(T, B, W, S, C)
    xbar = xw.mean(axis=(0, 3))                      # [B, W, C]
    q_reg = xbar @ w_qkv[:, :C] + b_qkv[:C]
    k_reg = xbar @ w_qkv[:, C:2 * C] + b_qkv[C:2 * C]
    a_r = np.einsum('bwc,bvc->bwv', q_reg, k_reg)
    routing_idx = np.argsort(-a_r, axis=-1)[:, :, :TOPK]   # [B, W, TOPK]
    return xw, routing_idx


def _hi_lo16(a):
    hi = a.astype(np.float16)
    lo = (a - hi.astype(np.float32)).astype(np.float16)
    return hi, lo


def _hi_lo_bf(a):
    hi = a.astype(ml_dtypes.bfloat16)
    lo = (a - hi.astype(np.float32)).astype(ml_dtypes.bfloat16)
    return hi, lo


def kernel(x, w_qkv, b_qkv, w_proj, b_proj):
    x = np.ascontiguousarray(np.asarray(x, dtype=np.float32))
    w_qkv = np.asarray(w_qkv, dtype=np.float32)
    b_qkv = np.asarray(b_qkv, dtype=np.float32)
    w_proj = np.asarray(w_proj, dtype=np.float32)
    b_proj = np.asarray(b_proj, dtype=np.float32)

    xw, routing_idx = _host_prepost(x, w_qkv, b_qkv)

    key = (routing_idx.tobytes(), QKV_MODE, QE_MODE, GE_ENG)
    if key not in _prog_cache:
        _prog_cache.clear()
        _prog_cache[key] = _build_program(routing_idx)
    nc = _prog_cache[key]

    split_x = QKV_MODE in ("f16x1", "f16x2", "f16x3")
    NSPX = 2 if QKV_MODE in ("f16x2", "f16x3") else 1
    NSPW = 2 if QKV_MODE == "f16x3" else 1
    np_qkv = np.float16 if split_x else np.float32
    tscale = np.array([2.0 ** (t - 1) for t in range(T)], np.float32)

    # weights (shared across cores)
    wkv = w_qkv[:, C:].astype(np.float32)
    wq = w_qkv[:, :C].astype(np.float32)
    if NSPW == 2:
        wkv_hi, wkv_lo = _hi_lo16(wkv)
        wkv_arr = np.stack([wkv_hi, wkv_lo]).reshape(2, 2, 128, 512)
        wq_hi, wq_lo = _hi_lo16(wq)
        wq_arr = np.stack([wq_hi, wq_lo]).reshape(2, 2, 128, 256)
    else:
        wkv_arr = wkv.reshape(1, 2, 128, 512)
        wq_arr = wq.reshape(1, 2, 128, 256)

    bkv_arr = np.empty((T, 2, 512), dtype=ml_dtypes.bfloat16)
    bq_arr = np.empty((T, 2, 256), dtype=ml_dtypes.bfloat16)
    for t in range(T):
        hi, lo = _hi_lo_bf(tscale[t] * b_qkv[C:])
        bkv_arr[t, 0], bkv_arr[t, 1] = hi, lo
        hi, lo = _hi_lo_bf(tscale[t] * b_qkv[:C])
        bq_arr[t, 0], bq_arr[t, 1] = hi, lo

    wp = (SCALE * w_proj).reshape(2, 128, 256).astype(np.float16)
    bp = b_proj.reshape(2, 128, 1).astype(np.float32)

    in_maps = []
    for r in range(NCORES):
        b_of, wg = r // 4, r % 4
        xwc = xw[:, b_of, wg * 8:(wg + 1) * 8]              # [T, 8, S, C]
        xl32 = np.ascontiguousarray(
            xwc.transpose(1, 3, 0, 2))                      # [NW, C, T, S]
        xl32 = xl32 * tscale[None, None, :, None]
        xl32 = xl32.reshape(NW, 2, 128, NTOK)
        if split_x:
            xhi = xl32.astype(np.float16)
            xlo = (xl32 - xhi.astype(np.float32)).astype(np.float16)
            xl = np.stack([xhi, xlo], axis=1)[:, :NSPX]     # [NW, NSPX, 2, 128, NTOK]
        else:
            xl = xl32.reshape(NW, 1, 2, 128, NTOK)
        in_maps.append({
            "x_in": np.ascontiguousarray(xl.astype(np_qkv)),
            "wkv_in": wkv_arr.astype(np_qkv), "wq_in": wq_arr.astype(np_qkv),
            "bkv_in": bkv_arr, "bq_in": bq_arr,
            "wp_in": wp, "bp_in": bp,
        })

    from concourse.bass_utils import run_bass_kernel_spmd
    res = run_bass_kernel_spmd(nc, in_maps, core_ids=list(range(NCORES)))

    # assemble output
    yw = np.empty((T, B, W, S, C), dtype=np.float32)
    for r in range(NCORES):
        b_of, wg = r // 4, r % 4
        o = res.results[r]["out_d"]                          # [NW, 2, 128, NTOK]
        o = o.reshape(NW, 2, 128, T, S).transpose(0, 3, 4, 1, 2).reshape(NW, T, S, C)
        for wl in range(NW):
            yw[:, b_of, wg * 8 + wl] = o[wl]

    y = yw.reshape(T, B, WT, WH, WW, GT, GH, GW, C) \
          .transpose(0, 1, 2, 5, 3, 6, 4, 7, 8).reshape(T, B, Lt, Lh, Lw, C)
    return y


# revision 24
# speedup vs baseline: 1.4120x; 1.0038x over previous
"""BiLevelRoutingAttention Trainium2 kernel (8-core SPMD), v2.

Sharding: core r handles batch b = r//4 and windows w in [ (r%4)*8, (r%4)*8+8 ).
Routing (region top-k) is computed on host via linearity of the mean.

Key design points vs v1:
- LIF runs in-place in PSUM with an exact 2^t rescaling of the membrane
  potential: g_t = 2^t * h_t.  Matmuls for step t accumulate 2^(t-1)*x_t@W
  onto the same PSUM bank (start=False), the spike is one is_ge(g, 2^t)
  (threshold scalars 1,2,4,8 are exact), and the hard reset is one
  copy_predicated(psum, spike, zeros) zeroing spiked lanes in place.  No
  h/v state tensors, no add/lt/mult chain.
- q is computed in phase 1 fused with k,v on the same x tile (x loaded once,
  rolling in SBUF); q spikes persist per window for phase 2.
- Attention is token-major: out^T[tok,e] = sq^T @ kv puts 128 tokens on the
  output partitions (PE cost ~ free size only), followed by a cheap f16 PE
  transpose back to feature-major for the projection.
- Spikes are binary so R = k^T v and kv sums are exact integer math in f16
  (counts <= 2048 exact).  attn counts can exceed 2048; f16 rounding there
  only feeds the final linear projection.

QKV matmul precision (KQKV env): "f16x2" (default: x hi+lo f16, w single
f16), "f16x3" (x hi+lo, w hi+lo, 3 cross terms), "float32r" (1 pass).
Q-spike engine (KQE env): "pool" (gpsimd, exact), "dvef16" (DVE, f16 out),
"dvef32r" (DVE, f32r out; q spikes stay 4-byte, attn lhsT f32r).
KV-spike engine (KGE env): "gpsimd" (default; DVE is_ge with fp16 output is
numerically wrong on HW).
"""

import numpy as np
import ml_dtypes
import os as _os

# ---- problem constants (hardcoded per contract) ----
T, B, Lt, Lh, Lw, C = 4, 2, 8, 32, 32, 256
WT, WH, WW = 2, 4, 4
W = WT * WH * WW            # 32 windows
GT, GH, GW = Lt // WT, Lh // WH, Lw // WW
S = GT * GH * GW            # 256 tokens per window
H, D = 8, C // 8            # 8 heads, 32 dim
TOPK = 8
SCALE = float(D) ** -0.5
NCORES = 8
NW = 8                      # windows per core
NTOK = T * S                # 1024 token-instances per window

QKV_MODE = _os.environ.get("KQKV", "f16x1")    # f16x1 | f16x2 | f16x3 | float32r
QE_MODE = _os.environ.get("KQE", "actpool")    # actpool | dvef16
GE_ENG = _os.environ.get("KGE", "dve")         # kv spike engine: dve (PSUM-capable)

_prog_cache = {}


def _split_sync_waits(nc, mybir, maxw=1):
    """walrus in this container rejects >1 sync wait per instruction; split
    excess waits onto NoOp instructions inserted just before."""
    for bb in nc.main_func.blocks:
        new_list = []
        for ins in bb.instructions:
            si = ins.sync_info
            waits = list(si.on_wait) if si is not None and si.on_wait else []
            if len(waits) > maxw:
                extra = waits[:-maxw]
                keep = waits[-maxw:]
                idx = 0
                while extra:
                    chunk, extra = extra[:maxw], extra[maxw:]
                    nop = mybir.InstNoOp(name=f"{ins.name}-wsplit{idx}", ins=[], outs=[])
                    nop.engine = ins.engine
                    nop.sync_info = mybir.SyncInfo(on_wait=chunk, on_update=[])
                    new_list.append(nop)
                    idx += 1
                ins.sync_info = mybir.SyncInfo(
                    on_wait=keep,
                    on_update=list(si.on_update) if si.on_update else [],
                )
            new_list.append(ins)
        bb.instructions[:] = new_list


def _build_program(routing_idx, sim_mode=False):
    """routing_idx: [B, W, TOPK] int array (host-computed). Returns nc.
    sim_mode: no collective / no If-chain (single-core TimelineSim)."""
    import contextlib
    import concourse.bass as bass
    import concourse.mybir as mybir
    import concourse.tile as tile

    f32 = mybir.dt.float32
    f32r = mybir.dt.float32r
    f16 = mybir.dt.float16
    bf16 = mybir.dt.bfloat16
    ALU = mybir.AluOpType
    ACT = mybir.ActivationFunctionType
    def reset_mask(ps_ap, g_ap, thr):
        # ps = (g < thr) * g; g is the SBUF snapshot of ps (exact), so only
        # the output touches PSUM (HW: one PSUM operand per DVE op)
        nc.vector.scalar_tensor_tensor(ps_ap, g_ap, thr, g_ap,
                                       ALU.is_lt, ALU.mult)

    split_x = QKV_MODE in ("f16x1", "f16x2", "f16x3")
    NSPX = 2 if QKV_MODE in ("f16x2", "f16x3") else 1
    NSPW = 2 if QKV_MODE == "f16x3" else 1
    if QKV_MODE == "f16x1":
        PASSES = ((0, 0),)
    elif QKV_MODE == "f16x2":
        PASSES = ((0, 0), (1, 0))
    elif QKV_MODE == "f16x3":
        PASSES = ((0, 0), (0, 1), (1, 0))
    else:
        PASSES = ((0, 0),)
    qkv_dt = f16 if split_x else f32r
    sq_dt = f32r if QE_MODE == "dvef32r" else f16

    nc = bass.Bass(num_devices=NCORES)
    ge_eng = nc.vector if GE_ENG == "dve" else getattr(nc, GE_ENG)
    qe_eng = nc.vector

    # ---- I/O ----
    # x feature-major, per-t pre-scaled by 2^(t-1): [wi, xsplit, kc, c(128), (t,s)]
    x_in = nc.dram_tensor("x_in", [NW, NSPX, 2, 128, NTOK], qkv_dt, kind="ExternalInput")
    wkv_in = nc.dram_tensor("wkv_in", [NSPW, 2, 128, 512], qkv_dt, kind="ExternalInput")
    wq_in = nc.dram_tensor("wq_in", [NSPW, 2, 128, 256], qkv_dt, kind="ExternalInput")
    # bias rows pre-scaled by 2^(t-1), bf16 hi+lo splits: [t, (hi,lo), feat]
    bkv_in = nc.dram_tensor("bkv_in", [T, 2, 512], bf16, kind="ExternalInput")
    bq_in = nc.dram_tensor("bq_in", [T, 2, 256], bf16, kind="ExternalInput")
    wp_in = nc.dram_tensor("wp_in", [2, 128, 256], f16, kind="ExternalInput")
    bp_in = nc.dram_tensor("bp_in", [2, 128, 1], f32, kind="ExternalInput")
    out_d = nc.dram_tensor("out_d", [NW, 2, 128, NTOK], f32, kind="ExternalOutput")
    DBG = _os.environ.get("KDBG") == "1"
    ABL = _os.environ.get("KABL", "")
    if DBG:
        dbg_skv = nc.dram_tensor("dbg_skv", [NW, 128, 4096], f16, kind="ExternalOutput")
        dbg_sq = nc.dram_tensor("dbg_sq", [NW, 128, 2048],
                                f16 if QE_MODE != "dvef32r" else mybir.dt.float32,
                                kind="ExternalOutput")
        dbg_rloc = nc.dram_tensor("dbg_rloc", [128, 2048], f16, kind="ExternalOutput")
        dbg_kvw = nc.dram_tensor("dbg_kvw", [NW, 128, 256], f16, kind="ExternalOutput")

    with tile.TileContext(nc) as tc:
        with (
            tc.tile_pool(name="const", bufs=1) as constp,
            tc.tile_pool(name="xin", bufs=4) as xin_p,
            tc.tile_pool(name="skv", bufs=4) as skv_p,
            tc.tile_pool(name="gq", bufs=3) as gq_p,
            tc.tile_pool(name="persist", bufs=1) as pers_p,
            tc.tile_pool(name="attn", bufs=3) as attn_p,
            tc.tile_pool(name="outs", bufs=3) as out_p,
            tc.tile_pool(name="dram", bufs=1, space="DRAM") as dram_p,
        ):
            # ---- constants / weights ----
            wkv_sb = constp.tile([128, NSPW * 2 * 512], qkv_dt)
            wq_sb = constp.tile([128, NSPW * 2 * 256], qkv_dt)
            for sw in range(NSPW):
                for kc in range(2):
                    nc.sync.dma_start(
                        wkv_sb[:, (sw * 2 + kc) * 512:(sw * 2 + kc + 1) * 512],
                        wkv_in[sw, kc])
                    nc.sync.dma_start(
                        wq_sb[:, (sw * 2 + kc) * 256:(sw * 2 + kc + 1) * 256],
                        wq_in[sw, kc])
            wp_sb = constp.tile([128, 2 * 256], f16)
            for kc in range(2):
                nc.sync.dma_start(wp_sb[:, kc * 256:(kc + 1) * 256], wp_in[kc])
            bp_sb = constp.tile([128, 2], f32)
            for cft in range(2):
                nc.sync.dma_start(bp_sb[:, cft:cft + 1], bp_in[cft])
            bkv_sb = constp.tile([2, T * 512], bf16)     # rows (hi, lo) per t
            bq_sb = constp.tile([2, T * 256], bf16)
            for t in range(T):
                nc.sync.dma_start(bkv_sb[:, t * 512:(t + 1) * 512], bkv_in[t])
                nc.sync.dma_start(bq_sb[:, t * 256:(t + 1) * 256], bq_in[t])
            ones128 = constp.tile([2, 128], bf16)
            nc.vector.memset(ones128[:], 1.0)
            ones256 = constp.tile([2, 256], bf16)
            nc.vector.memset(ones256[:], 1.0)
            zeros = constp.tile([128, 1024], f32)
            nc.vector.memset(zeros[:], 0.0)

            # persistent across phases
            r_loc = pers_p.tile([128, 2048], f16)          # local R, (wi,slab,t,e)
            r_rk = [pers_p.tile([128, 2048], f16, name=f"rrk{i}") for i in range(4)]
            kv_w = [pers_p.tile([128, 256], f16, name=f"kvw{i}") for i in range(NW)]
            sq_w = [pers_p.tile([128, 2048], sq_dt, name=f"sqw{i}") for i in range(NW)]

            # ============ phase 1: qkv matmuls + in-place PSUM LIF + R ==========
            ph1 = contextlib.ExitStack()
            ps_kv = ph1.enter_context(tc.tile_pool(name="pskv", bufs=2, space="PSUM"))
            ps_q = ph1.enter_context(tc.tile_pool(name="psq", bufs=2, space="PSUM"))
            ps_r = ph1.enter_context(tc.tile_pool(name="psr", bufs=2, space="PSUM"))

            def xsl(x_sb, sp, kc, lo, hi):
                return x_sb[:, (sp * 2 + kc) * NTOK + lo:(sp * 2 + kc) * NTOK + hi]

            rb_in = [dram_p.tile([128, 1536], f16, name="rbin0"),
                     dram_p.tile([128, 512], f16, name="rbin1")]
            rb_out = [dram_p.tile([4, 128, 1536], f16, name="rbout0"),
                      dram_p.tile([4, 128, 512], f16, name="rbout1")]

            def exchange(half):
                lo, hi = (0, 1536) if half == 0 else (1536, 2048)
                nc.sync.dma_start(rb_in[half][:], r_loc[:, lo:hi])
                if sim_mode:
                    for rk in range(4):
                        nc.sync.dma_start(rb_out[half][rk], rb_in[half][:])
                else:
                    nc.gpsimd.collective_compute(
                        "AllGather",
                        mybir.AluOpType.bypass,
                        replica_groups=[[0, 1, 2, 3], [4, 5, 6, 7]],
                        ins=[rb_in[half][:].opt()],
                        outs=[rb_out[half][:].opt()],
                    )
                for rk in range(4):
                    nc.sync.dma_start(r_rk[rk][:, lo:hi], rb_out[half][rk])

            pending_R = []
            emit_R_fns = [None]

            def emit_pending():
                emit_R_fns[0](*pending_R.pop(0))

            for wpair in range(NW // 2):
                if wpair == 3:
                    while pending_R:     # windows 4,5 R before their exchange
                        emit_pending()
                    exchange(0)          # windows 0-5 ready; overlaps pair 3
                ws = [2 * wpair, 2 * wpair + 1]
                x_t = {}
                pkv = {}
                pq = {}
                skv = {}
                started = {}
                for wi in ws:
                    x_sb = xin_p.tile([128, NSPX * 2 * NTOK], qkv_dt, tag="xsb")
                    for sp in range(NSPX):
                        for kc in range(2):
                            nc.sync.dma_start(
                                x_sb[:, (sp * 2 + kc) * NTOK:(sp * 2 + kc + 1) * NTOK],
                                x_in[wi, sp, kc])
                    x_t[wi] = x_sb
                    pkv[wi] = ps_kv.tile([128, 1024], f32, tag="pkv", name=f"pkv{wi}")
                    pq[wi] = ps_q.tile([128, 512], f32, tag="pq", name=f"pq{wi}")
                    skv[wi] = skv_p.tile([128, 8 * 512], f16, tag="skv", name=f"skv{wi}")
                    # one flag per PSUM zero region (2KB): kv sh0, kv sh1, q
                    started[wi] = [False, False, False]

                def mm(wi, bank, reg, lhsT, rhs):
                    first = not started[wi][bank]
                    started[wi][bank] = True
                    nc.tensor.matmul(reg, lhsT=lhsT, rhs=rhs, start=first,
                                     stop=first, skip_group_check=not first)

                for t in range(T):
                    # ---- kv matmuls (token-major): out [s-half, 512 feat] ----
                    for wi in ws:
                        for sh in range(2):
                            st = t * 2 + sh
                            reg = pkv[wi][:, sh * 512:(sh + 1) * 512]
                            for kc in range(2):
                                for (sx, sw) in PASSES:
                                    mm(wi, sh, reg,
                                       xsl(x_t[wi], sx, kc, st * 128, (st + 1) * 128),
                                       wkv_sb[:, (sw * 2 + kc) * 512:(sw * 2 + kc + 1) * 512])
                            mm(wi, sh, reg, ones128[:],
                               bkv_sb[:, t * 512:(t + 1) * 512])
                    # ---- kv spike + in-place reset ----
                    for wi in ws:
                        sksl = skv[wi][:, (t * 2) * 512:(t * 2 + 2) * 512]
                        if t < T - 1:
                            gkv = gq_p.tile([128, 1024], f32, tag="gkv",
                                            name=f"gkv{wi}_{t}")
                            nc.scalar.activation(gkv[:], pkv[wi][:], ACT.Copy,
                                                 bias=0.0, scale=1.0)
                            reset_mask(pkv[wi][:], gkv[:], float(2 ** t))
                            nc.gpsimd.tensor_scalar(sksl, gkv[:], float(2 ** t),
                                                    None, ALU.is_ge)
                        else:
                            nc.vector.tensor_scalar(sksl, pkv[wi][:],
                                                    float(2 ** t), None, ALU.is_ge)
                    # ---- q matmuls (feature-major): out [feat-half, 256 tok] ----
                    for wi in ws:
                        for ftc in range(2):
                            reg = pq[wi][:, ftc * 256:(ftc + 1) * 256]
                            for kc in range(2):
                                for (sx, sw) in PASSES:
                                    mm(wi, 2, reg,
                                       wq_sb[:, (sw * 2 + kc) * 256 + ftc * 128:
                                             (sw * 2 + kc) * 256 + (ftc + 1) * 128],
                                       xsl(x_t[wi], sx, kc, t * 256, (t + 1) * 256))
                            mm(wi, 2, reg,
                               bq_sb[:, t * 256 + ftc * 128:t * 256 + (ftc + 1) * 128],
                               ones256[:])
                    # ---- q spike + in-place reset ----
                    for wi in ws:
                        sqsl = sq_w[wi][:, t * 512:(t + 1) * 512]
                        if t < T - 1:
                            gq = gq_p.tile([128, 512], f32, tag="gq",
                                           name=f"gq{wi}_{t}")
                            nc.scalar.activation(gq[:], pq[wi][:], ACT.Copy,
                                                 bias=0.0, scale=1.0)
                            reset_mask(pq[wi][:], gq[:], float(2 ** t))
                            qeng = nc.gpsimd if wi % 2 == 0 else nc.vector
                            qeng.tensor_scalar(sqsl, gq[:], float(2 ** t),
                                               None, ALU.is_ge)
                        else:
                            nc.vector.tensor_scalar(sqsl, pq[wi][:],
                                                    float(2 ** t), None, ALU.is_ge)

                # ---- R = k^T v per (t, head): [d,e] blocks, col-tiled 4 heads ----
                def emit_R(wi, skv_t):
                    for slab in range(2):
                        psr = ps_r.tile([128, 128], f32, tag="psr",
                                        name=f"psr{wi}{slab}")
                        for t in range(T):
                            for hl in range(4):
                                h = slab * 4 + hl
                                for sh in range(2):
                                    st = t * 2 + sh
                                    nc.tensor.matmul(
                                        psr[32 * hl:32 * (hl + 1), t * 32:(t + 1) * 32],
                                        lhsT=skv_t[:, st * 512 + h * 32:
                                                   st * 512 + (h + 1) * 32],
                                        rhs=skv_t[:, st * 512 + 256 + h * 32:
                                                  st * 512 + 256 + (h + 1) * 32],
                                        start=(sh == 0), stop=(sh == 1),
                                        tile_position=(0, 32 * hl),
                                    )
                        r_view = r_loc[:].rearrange(
                            "p (w a t e) -> p w a t e", w=8, a=2, t=4, e=32)
                        nc.scalar.activation(
                            r_view[:, wi, slab, :, :],
                            psr[:].rearrange("p (t e) -> p t e", t=4, e=32),
                            ACT.Copy, bias=0.0, scale=1.0)
                    if DBG:
                        nc.sync.dma_start(dbg_skv[wi], skv_t[:])
                        nc.sync.dma_start(dbg_sq[wi], sq_w[wi][:])

                emit_R_fns[0] = emit_R
                for wi in ws:
                    pending_R.append((wi, skv[wi]))
                while len(pending_R) > 2:
                    emit_pending()
                if wpair == NW // 2 - 1:
                    while pending_R:
                        emit_pending()

            ph1.close()
            # ============ phase 2: finish exchange, kv sums, attention, proj ====
            ph2 = contextlib.ExitStack()
            ps_at = ph2.enter_context(tc.tile_pool(name="psat", bufs=3, space="PSUM"))
            ps_pj = ph2.enter_context(tc.tile_pool(name="pspj", bufs=3, space="PSUM"))
            exchange(1)

            # kv sums: routed gather baked per core, guarded by If on core id
            pid = None if sim_mode else nc.partition_id()
            for r in ([] if ABL == "ph1" else range(NCORES)):
                if sim_mode and r != 0:
                    continue
                b_of = r // 4
                wg = r % 4
                with (contextlib.nullcontext() if sim_mode else tc.If(pid == r)):
                    for wl in range(NW):
                        wglob = wg * 8 + wl
                        idxs = [int(j) for j in routing_idx[b_of, wglob]]
                        idxs.sort(key=lambda j: (j % 8) >= 6)
                        eng = nc.vector if (wl % 2 == 0) else nc.gpsimd

                        def rsrc(j):
                            return r_rk[j // 8][:, (j % 8) * 256:(j % 8 + 1) * 256]
                        dst = kv_w[wl][:]
                        eng.tensor_copy(dst, rsrc(idxs[0]))
                        for j in idxs[1:]:
                            eng.tensor_tensor(dst, dst, rsrc(j), op=ALU.add)

            if DBG:
                nc.sync.dma_start(dbg_rloc[:], r_loc[:])
                for wl in range(NW):
                    nc.sync.dma_start(dbg_kvw[wl], kv_w[wl][:])
            # ---- feature-major attention + projection per window ----
            for wl in ([] if ABL == "ph1" else range(NW)):
                # attn feature-major: [e-feat(128 of slab), (slab, t, tok)]
                attn_sb = attn_p.tile([128, 2 * NTOK], f16, tag="attn")
                attn_v = attn_sb[:].rearrange("p (c t e) -> p c t e",
                                              c=2, t=4, e=256)
                outsb = out_p.tile([128, 2048], f32, tag="outsb")
                for nch in range(2):
                    for t in (2 * nch, 2 * nch + 1):
                        psa = ps_at.tile([128, 512], f32, tag="psat")
                        for slab in range(2):
                            for hl in range(4):
                                nc.tensor.matmul(
                                    psa[32 * hl:32 * (hl + 1),
                                        slab * 256:(slab + 1) * 256],
                                    lhsT=kv_w[wl][32 * hl:32 * (hl + 1),
                                                  (slab * 4 + t) * 32:
                                                  (slab * 4 + t + 1) * 32],
                                    rhs=sq_w[wl][32 * hl:32 * (hl + 1),
                                                 (t * 2 + slab) * 256:
                                                 (t * 2 + slab + 1) * 256],
                                    start=True, stop=True,
                                    tile_position=(32 * hl, 32 * hl),
                                )
                        nc.vector.tensor_copy(
                            attn_v[:, :, t, :],
                            psa[:].rearrange("p (c e) -> p c e", c=2, e=256))
                    for cft in range(2):
                        psp = ps_pj.tile([128, 512], f32, tag="pspj")
                        for kc in range(2):
                            nc.tensor.matmul(
                                psp[:],
                                lhsT=wp_sb[:, kc * 256 + cft * 128:
                                           kc * 256 + (cft + 1) * 128],
                                rhs=attn_sb[:, kc * 1024 + nch * 512:
                                            kc * 1024 + (nch + 1) * 512],
                                start=(kc == 0), stop=(kc == 1),
                            )
                        nc.scalar.activation(
                            outsb[:, cft * NTOK + nch * 512:cft * NTOK + (nch + 1) * 512],
                            psp[:], ACT.Identity, bias=bp_sb[:, cft:cft + 1], scale=1.0)
                        nc.sync.dma_start(
                            out_d[wl, cft, :, nch * 512:(nch + 1) * 512],
                            outsb[:, cft * NTOK + nch * 512:cft * NTOK + (nch + 1) * 512])
            ph2.close()

    _split_sync_waits(nc, mybir, maxw=1)
    return nc


def _host_prepost(x, w_qkv, b_qkv):
    """Window partition, routing."""
    xw = x.reshape(T, B, WT, GT, WH, GH, WW, GW, C) \
          .transpose(0, 1, 2, 4, 6, 3, 5, 7, 8).reshape(T, B, W, S, C)
    xbar = xw.mean(axis=(0, 3))                      # [B, W, C]
    q_reg = xbar @ w_qkv[:, :C] + b_qkv[:C]
    k_reg = xbar @ w_qkv[:, C:2 * C] + b_qkv[C:2 * C]
    a_r = np.einsum('bwc,bvc->bwv', q_reg, k_reg)
    routing_idx = np.argsort(-a_r, axis=-1)[:, :, :TOPK]   # [B, W, TOPK]
    return xw, routing_idx


def _hi_lo16(a):
    hi = a.astype(np.float16)
    lo = (a - hi.astype(np.float32)).astype(np.float16)
    return hi, lo


def _hi_lo_bf(a):
    hi = a.astype(ml_dtypes.bfloat16)
    lo = (a - hi.astype(np.float32)).astype(ml_dtypes.bfloat16)
    return hi, lo


def kernel(x, w_qkv, b_qkv, w_proj, b_proj):
    x = np.ascontiguousarray(np.asarray(x, dtype=np.float32))
    w_qkv = np.asarray(w_qkv, dtype=np.float32)
    b_qkv = np.asarray(b_qkv, dtype=np.float32)
    w_proj = np.asarray(w_proj, dtype=np.float32)
    b_proj = np.asarray(b_proj, dtype=np.float32)

    xw, routing_idx = _host_prepost(x, w_qkv, b_qkv)

    key = (routing_idx.tobytes(), QKV_MODE, QE_MODE, GE_ENG)
    if key not in _prog_cache:
        _prog_cache.clear()
        _prog_cache[key] = _build_program(routing_idx)
    nc = _prog_cache[key]

    split_x = QKV_MODE in ("f16x1", "f16x2", "f16x3")
    NSPX = 2 if QKV_MODE in ("f16x2", "f16x3") else 1
    NSPW = 2 if QKV_MODE == "f16x3" else 1
    np_qkv = np.float16 if split_x else np.float32
    tscale = np.array([2.0 ** (t - 1) for t in range(T)], np.float32)

    # weights (shared across cores)
    wkv = w_qkv[:, C:].astype(np.float32)
    wq = w_qkv[:, :C].astype(np.float32)
    if NSPW == 2:
        wkv_hi, wkv_lo = _hi_lo16(wkv)
        wkv_arr = np.stack([wkv_hi, wkv_lo]).reshape(2, 2, 128, 512)
        wq_hi, wq_lo = _hi_lo16(wq)
        wq_arr = np.stack([wq_hi, wq_lo]).reshape(2, 2, 128, 256)
    else:
        wkv_arr = wkv.reshape(1, 2, 128, 512)
        wq_arr = wq.reshape(1, 2, 128, 256)

    bkv_arr = np.empty((T, 2, 512), dtype=ml_dtypes.bfloat16)
    bq_arr = np.empty((T, 2, 256), dtype=ml_dtypes.bfloat16)
    for t in range(T):
        hi, lo = _hi_lo_bf(tscale[t] * b_qkv[C:])
        bkv_arr[t, 0], bkv_arr[t, 1] = hi, lo
        hi, lo = _hi_lo_bf(tscale[t] * b_qkv[:C])
        bq_arr[t, 0], bq_arr[t, 1] = hi, lo

    wp = (SCALE * w_proj).reshape(2, 128, 256).astype(np.float16)
    bp = b_proj.reshape(2, 128, 1).astype(np.float32)

    in_maps = []
    for r in range(NCORES):
        b_of, wg = r // 4, r % 4
        xwc = xw[:, b_of, wg * 8:(wg + 1) * 8]              # [T, 8, S, C]
        xl32 = np.ascontiguousarray(
            xwc.transpose(1, 3, 0, 2))                      # [NW, C, T, S]
        xl32 = xl32 * tscale[None, None, :, None]
        xl32 = xl32.reshape(NW, 2, 128, NTOK)
        if split_x:
            xhi = xl32.astype(np.float16)
            xlo = (xl32 - xhi.astype(np.float32)).astype(np.float16)
            xl = np.stack([xhi, xlo], axis=1)[:, :NSPX]     # [NW, NSPX, 2, 128, NTOK]
        else:
            xl = xl32.reshape(NW, 1, 2, 128, NTOK)
        in_maps.append({
            "x_in": np.ascontiguousarray(xl.astype(np_qkv)),
            "wkv_in": wkv_arr.astype(np_qkv), "wq_in": wq_arr.astype(np_qkv),
            "bkv_in": bkv_arr, "bq_in": bq_arr,
            "wp_in": wp, "bp_in": bp,
        })

    from concourse.bass_utils import run_bass_kernel_spmd
    res = run_bass_kernel_spmd(nc, in_maps, core_ids=list(range(NCORES)))

    # assemble output
    yw = np.empty((T, B, W, S, C), dtype=np.float32)
    for r in range(NCORES):
        b_of, wg = r // 4, r % 4
        o = res.results[r]["out_d"]                          # [NW, 2, 128, NTOK]
        o = o.reshape(NW, 2, 128, T, S).transpose(0, 3, 4, 1, 2).reshape(NW, T, S, C)
        for wl in range(NW):
            yw[:, b_of, wg * 8 + wl] = o[wl]

    y = yw.reshape(T, B, WT, WH, WW, GT, GH, GW, C) \
          .transpose(0, 1, 2, 5, 3, 6, 4, 7, 8).reshape(T, B, Lt, Lh, Lw, C)
    return y


# revision 26
# speedup vs baseline: 1.4135x; 1.0010x over previous
"""BiLevelRoutingAttention Trainium2 kernel (8-core SPMD), v2.

Sharding: core r handles batch b = r//4 and windows w in [ (r%4)*8, (r%4)*8+8 ).
Routing (region top-k) is computed on host via linearity of the mean.

Key design points vs the v1 baseline (242.7us -> ~152.7us TimelineSim):
- QKV matmul runs single-pass f16 (KQKV=f16x1): measured end-to-end error
  is identical to the 2-pass hi/lo split (1.16e-3, w-rounding dominated),
  at half the PE time and half the x DMA.
- LIF runs in-place in PSUM with an exact 2^t rescaling of the membrane
  potential: g_t = 2^t * h_t (x pre-scaled by 2^(t-1) on host, thresholds
  1,2,4,8 exact).  Matmuls for step t accumulate onto the same PSUM bank
  (start=False, skip_group_check); the hard reset is one DVE
  scalar_tensor_tensor: psum = (g_snap < 2^t) * g_snap, where g_snap is an
  Act-engine SBUF snapshot (DVE may touch PSUM with only one operand).
  Spikes are thresholded from the same snapshot on Pool/DVE off the
  critical path, exactly consistent with the reset.  No h/v state tensors.
- q is computed in phase 1 fused with k,v on the same x tile (x loaded
  once, rolling in SBUF); q spikes persist per window for phase 2.
- R = k^T v accumulates 4 timesteps into one [128,128] PSUM tile per
  (window, slab); R emission is deferred one window-pair so lazy spikes
  never stall the PE queue.
- The R AllGather is split: windows 0-5 exchange after pair 2 (overlapped
  with compute), windows 6-7 at phase-2 start; per-rank landing tiles
  keep the unpack DMAs contiguous.
- Attention is feature-major (baseline-proven quadrant matmuls); kv sums
  are exact integer f16 adds (counts <= 2048 exact), attn psum copies on
  DVE, projection in f16 with SCALE folded into w_proj.

Env toggles (debug/fallback): KQKV in {f16x1,f16x2,f16x3,float32r},
KQE/KGE spike-engine overrides, KDBG=1 debug dumps, KABL=ph1 ablation.
"""

import numpy as np
import ml_dtypes
import os as _os

# ---- problem constants (hardcoded per contract) ----
T, B, Lt, Lh, Lw, C = 4, 2, 8, 32, 32, 256
WT, WH, WW = 2, 4, 4
W = WT * WH * WW            # 32 windows
GT, GH, GW = Lt // WT, Lh // WH, Lw // WW
S = GT * GH * GW            # 256 tokens per window
H, D = 8, C // 8            # 8 heads, 32 dim
TOPK = 8
SCALE = float(D) ** -0.5
NCORES = 8
NW = 8                      # windows per core
NTOK = T * S                # 1024 token-instances per window

QKV_MODE = _os.environ.get("KQKV", "f16x1")    # f16x1 | f16x2 | f16x3 | float32r
QE_MODE = _os.environ.get("KQE", "actpool")    # actpool | dvef16
GE_ENG = _os.environ.get("KGE", "dve")         # kv spike engine: dve (PSUM-capable)

_prog_cache = {}


def _split_sync_waits(nc, mybir, maxw=1):
    """walrus in this container rejects >1 sync wait per instruction; split
    excess waits onto NoOp instructions inserted just before."""
    for bb in nc.main_func.blocks:
        new_list = []
        for ins in bb.instructions:
            si = ins.sync_info
            waits = list(si.on_wait) if si is not None and si.on_wait else []
            if len(waits) > maxw:
                extra = waits[:-maxw]
                keep = waits[-maxw:]
                idx = 0
                while extra:
                    chunk, extra = extra[:maxw], extra[maxw:]
                    nop = mybir.InstNoOp(name=f"{ins.name}-wsplit{idx}", ins=[], outs=[])
                    nop.engine = ins.engine
                    nop.sync_info = mybir.SyncInfo(on_wait=chunk, on_update=[])
                    new_list.append(nop)
                    idx += 1
                ins.sync_info = mybir.SyncInfo(
                    on_wait=keep,
                    on_update=list(si.on_update) if si.on_update else [],
                )
            new_list.append(ins)
        bb.instructions[:] = new_list


def _build_program(routing_idx, sim_mode=False):
    """routing_idx: [B, W, TOPK] int array (host-computed). Returns nc.
    sim_mode: no collective / no If-chain (single-core TimelineSim)."""
    import contextlib
    import concourse.bass as bass
    import concourse.mybir as mybir
    import concourse.tile as tile

    f32 = mybir.dt.float32
    f32r = mybir.dt.float32r
    f16 = mybir.dt.float16
    bf16 = mybir.dt.bfloat16
    ALU = mybir.AluOpType
    ACT = mybir.ActivationFunctionType
    def reset_mask(ps_ap, g_ap, thr):
        # ps = (g < thr) * g; g is the SBUF snapshot of ps (exact), so only
        # the output touches PSUM (HW: one PSUM operand per DVE op)
        nc.vector.scalar_tensor_tensor(ps_ap, g_ap, thr, g_ap,
                                       ALU.is_lt, ALU.mult)

    split_x = QKV_MODE in ("f16x1", "f16x2", "f16x3")
    NSPX = 2 if QKV_MODE in ("f16x2", "f16x3") else 1
    NSPW = 2 if QKV_MODE == "f16x3" else 1
    if QKV_MODE == "f16x1":
        PASSES = ((0, 0),)
    elif QKV_MODE == "f16x2":
        PASSES = ((0, 0), (1, 0))
    elif QKV_MODE == "f16x3":
        PASSES = ((0, 0), (0, 1), (1, 0))
    else:
        PASSES = ((0, 0),)
    qkv_dt = f16 if split_x else f32r
    sq_dt = f32r if QE_MODE == "dvef32r" else f16

    nc = bass.Bass(num_devices=NCORES)
    ge_eng = nc.vector if GE_ENG == "dve" else getattr(nc, GE_ENG)
    qe_eng = nc.vector

    # ---- I/O ----
    # x feature-major, per-t pre-scaled by 2^(t-1): [wi, xsplit, kc, c(128), (t,s)]
    x_in = nc.dram_tensor("x_in", [NW, NSPX, 2, 128, NTOK], qkv_dt, kind="ExternalInput")
    wkv_in = nc.dram_tensor("wkv_in", [NSPW, 2, 128, 512], qkv_dt, kind="ExternalInput")
    wq_in = nc.dram_tensor("wq_in", [NSPW, 2, 128, 256], qkv_dt, kind="ExternalInput")
    # bias rows pre-scaled by 2^(t-1), bf16 hi+lo splits: [t, (hi,lo), feat]
    bkv_in = nc.dram_tensor("bkv_in", [T, 2, 512], bf16, kind="ExternalInput")
    bq_in = nc.dram_tensor("bq_in", [T, 2, 256], bf16, kind="ExternalInput")
    wp_in = nc.dram_tensor("wp_in", [2, 128, 256], f16, kind="ExternalInput")
    bp_in = nc.dram_tensor("bp_in", [2, 128, 1], f32, kind="ExternalInput")
    out_d = nc.dram_tensor("out_d", [NW, 2, 128, NTOK], f32, kind="ExternalOutput")
    DBG = _os.environ.get("KDBG") == "1"
    ABL = _os.environ.get("KABL", "")
    if DBG:
        dbg_skv = nc.dram_tensor("dbg_skv", [NW, 128, 4096], f16, kind="ExternalOutput")
        dbg_sq = nc.dram_tensor("dbg_sq", [NW, 128, 2048],
                                f16 if QE_MODE != "dvef32r" else mybir.dt.float32,
                                kind="ExternalOutput")
        dbg_rloc = nc.dram_tensor("dbg_rloc", [128, 2048], f16, kind="ExternalOutput")
        dbg_kvw = nc.dram_tensor("dbg_kvw", [NW, 128, 256], f16, kind="ExternalOutput")

    with tile.TileContext(nc) as tc:
        with (
            tc.tile_pool(name="const", bufs=1) as constp,
            tc.tile_pool(name="xin", bufs=4) as xin_p,
            tc.tile_pool(name="skv", bufs=4) as skv_p,
            tc.tile_pool(name="gq", bufs=6) as gq_p,
            tc.tile_pool(name="persist", bufs=1) as pers_p,
            tc.tile_pool(name="attn", bufs=3) as attn_p,
            tc.tile_pool(name="outs", bufs=3) as out_p,
            tc.tile_pool(name="dram", bufs=1, space="DRAM") as dram_p,
        ):
            # ---- constants / weights ----
            wkv_sb = constp.tile([128, NSPW * 2 * 512], qkv_dt)
            wq_sb = constp.tile([128, NSPW * 2 * 256], qkv_dt)
            for sw in range(NSPW):
                for kc in range(2):
                    nc.sync.dma_start(
                        wkv_sb[:, (sw * 2 + kc) * 512:(sw * 2 + kc + 1) * 512],
                        wkv_in[sw, kc])
                    nc.sync.dma_start(
                        wq_sb[:, (sw * 2 + kc) * 256:(sw * 2 + kc + 1) * 256],
                        wq_in[sw, kc])
            wp_sb = constp.tile([128, 2 * 256], f16)
            for kc in range(2):
                nc.sync.dma_start(wp_sb[:, kc * 256:(kc + 1) * 256], wp_in[kc])
            bp_sb = constp.tile([128, 2], f32)
            for cft in range(2):
                nc.sync.dma_start(bp_sb[:, cft:cft + 1], bp_in[cft])
            bkv_sb = constp.tile([2, T * 512], bf16)     # rows (hi, lo) per t
            bq_sb = constp.tile([2, T * 256], bf16)
            for t in range(T):
                nc.sync.dma_start(bkv_sb[:, t * 512:(t + 1) * 512], bkv_in[t])
                nc.sync.dma_start(bq_sb[:, t * 256:(t + 1) * 256], bq_in[t])
            ones128 = constp.tile([2, 128], bf16)
            nc.vector.memset(ones128[:], 1.0)
            ones256 = constp.tile([2, 256], bf16)
            nc.vector.memset(ones256[:], 1.0)
            zeros = constp.tile([128, 1024], f32)
            nc.vector.memset(zeros[:], 0.0)

            # persistent across phases
            r_loc = pers_p.tile([128, 2048], f16)          # local R, (wi,slab,t,e)
            r_rk = [pers_p.tile([128, 2048], f16, name=f"rrk{i}") for i in range(4)]
            kv_w = [pers_p.tile([128, 256], f16, name=f"kvw{i}") for i in range(NW)]
            sq_w = [pers_p.tile([128, 2048], sq_dt, name=f"sqw{i}") for i in range(NW)]

            # ============ phase 1: qkv matmuls + in-place PSUM LIF + R ==========
            ph1 = contextlib.ExitStack()
            ps_kv = ph1.enter_context(tc.tile_pool(name="pskv", bufs=2, space="PSUM"))
            ps_q = ph1.enter_context(tc.tile_pool(name="psq", bufs=2, space="PSUM"))
            ps_r = ph1.enter_context(tc.tile_pool(name="psr", bufs=2, space="PSUM"))

            def xsl(x_sb, sp, kc, lo, hi):
                return x_sb[:, (sp * 2 + kc) * NTOK + lo:(sp * 2 + kc) * NTOK + hi]

            rb_in = [dram_p.tile([128, 1536], f16, name="rbin0"),
                     dram_p.tile([128, 512], f16, name="rbin1")]
            rb_out = [dram_p.tile([4, 128, 1536], f16, name="rbout0"),
                      dram_p.tile([4, 128, 512], f16, name="rbout1")]

            def exchange(half):
                lo, hi = (0, 1536) if half == 0 else (1536, 2048)
                nc.sync.dma_start(rb_in[half][:], r_loc[:, lo:hi])
                if sim_mode:
                    for rk in range(4):
                        nc.sync.dma_start(rb_out[half][rk], rb_in[half][:])
                else:
                    nc.gpsimd.collective_compute(
                        "AllGather",
                        mybir.AluOpType.bypass,
                        replica_groups=[[0, 1, 2, 3], [4, 5, 6, 7]],
                        ins=[rb_in[half][:].opt()],
                        outs=[rb_out[half][:].opt()],
                    )
                for rk in range(4):
                    nc.sync.dma_start(r_rk[rk][:, lo:hi], rb_out[half][rk])

            pending_R = []
            emit_R_fns = [None]

            def emit_pending():
                emit_R_fns[0](*pending_R.pop(0))

            for wpair in range(NW // 2):
                if wpair == 3:
                    while pending_R:     # windows 4,5 R before their exchange
                        emit_pending()
                    exchange(0)          # windows 0-5 ready; overlaps pair 3
                ws = [2 * wpair, 2 * wpair + 1]
                x_t = {}
                pkv = {}
                pq = {}
                skv = {}
                started = {}
                for wi in ws:
                    x_sb = xin_p.tile([128, NSPX * 2 * NTOK], qkv_dt, tag="xsb")
                    for sp in range(NSPX):
                        for kc in range(2):
                            nc.sync.dma_start(
                                x_sb[:, (sp * 2 + kc) * NTOK:(sp * 2 + kc + 1) * NTOK],
                                x_in[wi, sp, kc])
                    x_t[wi] = x_sb
                    pkv[wi] = ps_kv.tile([128, 1024], f32, tag="pkv", name=f"pkv{wi}")
                    pq[wi] = ps_q.tile([128, 512], f32, tag="pq", name=f"pq{wi}")
                    skv[wi] = skv_p.tile([128, 8 * 512], f16, tag="skv", name=f"skv{wi}")
                    # one flag per PSUM zero region (2KB): kv sh0, kv sh1, q
                    started[wi] = [False, False, False]

                def mm(wi, bank, reg, lhsT, rhs):
                    first = not started[wi][bank]
                    started[wi][bank] = True
                    nc.tensor.matmul(reg, lhsT=lhsT, rhs=rhs, start=first,
                                     stop=first, skip_group_check=not first)

                for t in range(T):
                    # ---- kv matmuls (token-major): out [s-half, 512 feat] ----
                    for wi in ws:
                        for sh in range(2):
                            st = t * 2 + sh
                            reg = pkv[wi][:, sh * 512:(sh + 1) * 512]
                            for kc in range(2):
                                for (sx, sw) in PASSES:
                                    mm(wi, sh, reg,
                                       xsl(x_t[wi], sx, kc, st * 128, (st + 1) * 128),
                                       wkv_sb[:, (sw * 2 + kc) * 512:(sw * 2 + kc + 1) * 512])
                            mm(wi, sh, reg, ones128[:],
                               bkv_sb[:, t * 512:(t + 1) * 512])
                    # ---- kv spike + in-place reset ----
                    for wi in ws:
                        sksl = skv[wi][:, (t * 2) * 512:(t * 2 + 2) * 512]
                        if t < T - 1:
                            gkv = gq_p.tile([128, 1024], f32, tag="gkv",
                                            name=f"gkv{wi}_{t}")
                            nc.scalar.activation(gkv[:], pkv[wi][:], ACT.Copy,
                                                 bias=0.0, scale=1.0)
                            reset_mask(pkv[wi][:], gkv[:], float(2 ** t))
                            nc.gpsimd.tensor_scalar(sksl, gkv[:], float(2 ** t),
                                                    None, ALU.is_ge)
                        else:
                            nc.vector.tensor_scalar(sksl, pkv[wi][:],
                                                    float(2 ** t), None, ALU.is_ge)
                    # ---- q matmuls (feature-major): out [feat-half, 256 tok] ----
                    for wi in ws:
                        for ftc in range(2):
                            reg = pq[wi][:, ftc * 256:(ftc + 1) * 256]
                            for kc in range(2):
                                for (sx, sw) in PASSES:
                                    mm(wi, 2, reg,
                                       wq_sb[:, (sw * 2 + kc) * 256 + ftc * 128:
                                             (sw * 2 + kc) * 256 + (ftc + 1) * 128],
                                       xsl(x_t[wi], sx, kc, t * 256, (t + 1) * 256))
                            mm(wi, 2, reg,
                               bq_sb[:, t * 256 + ftc * 128:t * 256 + (ftc + 1) * 128],
                               ones256[:])
                    # ---- q spike + in-place reset ----
                    for wi in ws:
                        sqsl = sq_w[wi][:, t * 512:(t + 1) * 512]
                        if t < T - 1:
                            gq = gq_p.tile([128, 512], f32, tag="gq",
                                           name=f"gq{wi}_{t}")
                            nc.scalar.activation(gq[:], pq[wi][:], ACT.Copy,
                                                 bias=0.0, scale=1.0)
                            reset_mask(pq[wi][:], gq[:], float(2 ** t))
                            qeng = nc.gpsimd if wi % 2 == 0 else nc.vector
                            qeng.tensor_scalar(sqsl, gq[:], float(2 ** t),
                                               None, ALU.is_ge)
                        else:
                            nc.vector.tensor_scalar(sqsl, pq[wi][:],
                                                    float(2 ** t), None, ALU.is_ge)

                # ---- R = k^T v per (t, head): [d,e] blocks, col-tiled 4 heads ----
                def emit_R(wi, skv_t):
                    for slab in range(2):
                        psr = ps_r.tile([128, 128], f32, tag="psr",
                                        name=f"psr{wi}{slab}")
                        for t in range(T):
                            for hl in range(4):
                                h = slab * 4 + hl
                                for sh in range(2):
                                    st = t * 2 + sh
                                    nc.tensor.matmul(
                                        psr[32 * hl:32 * (hl + 1), t * 32:(t + 1) * 32],
                                        lhsT=skv_t[:, st * 512 + h * 32:
                                                   st * 512 + (h + 1) * 32],
                                        rhs=skv_t[:, st * 512 + 256 + h * 32:
                                                  st * 512 + 256 + (h + 1) * 32],
                                        start=(sh == 0), stop=(sh == 1),
                                        tile_position=(0, 32 * hl),
                                    )
                        r_view = r_loc[:].rearrange(
                            "p (w a t e) -> p w a t e", w=8, a=2, t=4, e=32)
                        nc.scalar.activation(
                            r_view[:, wi, slab, :, :],
                            psr[:].rearrange("p (t e) -> p t e", t=4, e=32),
                            ACT.Copy, bias=0.0, scale=1.0)
                    if DBG:
                        nc.sync.dma_start(dbg_skv[wi], skv_t[:])
                        nc.sync.dma_start(dbg_sq[wi], sq_w[wi][:])

                emit_R_fns[0] = emit_R
                for wi in ws:
                    pending_R.append((wi, skv[wi]))
                while len(pending_R) > 2:
                    emit_pending()
                if wpair == NW // 2 - 1:
                    while pending_R:
                        emit_pending()

            ph1.close()
            # ============ phase 2: finish exchange, kv sums, attention, proj ====
            ph2 = contextlib.ExitStack()
            ps_at = ph2.enter_context(tc.tile_pool(name="psat", bufs=3, space="PSUM"))
            ps_pj = ph2.enter_context(tc.tile_pool(name="pspj", bufs=3, space="PSUM"))
            exchange(1)

            # kv sums: routed gather baked per core, guarded by If on core id
            pid = None if sim_mode else nc.partition_id()
            for r in ([] if ABL == "ph1" else range(NCORES)):
                if sim_mode and r != 0:
                    continue
                b_of = r // 4
                wg = r % 4
                with (contextlib.nullcontext() if sim_mode else tc.If(pid == r)):
                    for wl in range(NW):
                        wglob = wg * 8 + wl
                        idxs = [int(j) for j in routing_idx[b_of, wglob]]
                        idxs.sort(key=lambda j: (j % 8) >= 6)
                        eng = nc.vector if (wl % 2 == 0) else nc.gpsimd

                        def rsrc(j):
                            return r_rk[j // 8][:, (j % 8) * 256:(j % 8 + 1) * 256]
                        dst = kv_w[wl][:]
                        eng.tensor_copy(dst, rsrc(idxs[0]))
                        for j in idxs[1:]:
                            eng.tensor_tensor(dst, dst, rsrc(j), op=ALU.add)

            if DBG:
                nc.sync.dma_start(dbg_rloc[:], r_loc[:])
                for wl in range(NW):
                    nc.sync.dma_start(dbg_kvw[wl], kv_w[wl][:])
            # ---- feature-major attention + projection per window ----
            for wl in ([] if ABL == "ph1" else range(NW)):
                # attn feature-major: [e-feat(128 of slab), (slab, t, tok)]
                attn_sb = attn_p.tile([128, 2 * NTOK], f16, tag="attn")
                attn_v = attn_sb[:].rearrange("p (c t e) -> p c t e",
                                              c=2, t=4, e=256)
                outsb = out_p.tile([128, 2048], f32, tag="outsb")
                for nch in range(2):
                    for t in (2 * nch, 2 * nch + 1):
                        psa = ps_at.tile([128, 512], f32, tag="psat")
                        for slab in range(2):
                            for hl in range(4):
                                nc.tensor.matmul(
                                    psa[32 * hl:32 * (hl + 1),
                                        slab * 256:(slab + 1) * 256],
                                    lhsT=kv_w[wl][32 * hl:32 * (hl + 1),
                                                  (slab * 4 + t) * 32:
                                                  (slab * 4 + t + 1) * 32],
                                    rhs=sq_w[wl][32 * hl:32 * (hl + 1),
                                                 (t * 2 + slab) * 256:
                                                 (t * 2 + slab + 1) * 256],
                                    start=True, stop=True,
                                    tile_position=(32 * hl, 32 * hl),
                                )
                        nc.vector.tensor_copy(
                            attn_v[:, :, t, :],
                            psa[:].rearrange("p (c e) -> p c e", c=2, e=256))
                    for cft in range(2):
                        psp = ps_pj.tile([128, 512], f32, tag="pspj")
                        for kc in range(2):
                            nc.tensor.matmul(
                                psp[:],
                                lhsT=wp_sb[:, kc * 256 + cft * 128:
                                           kc * 256 + (cft + 1) * 128],
                                rhs=attn_sb[:, kc * 1024 + nch * 512:
                                            kc * 1024 + (nch + 1) * 512],
                                start=(kc == 0), stop=(kc == 1),
                            )
                        nc.scalar.activation(
                            outsb[:, cft * NTOK + nch * 512:cft * NTOK + (nch + 1) * 512],
                            psp[:], ACT.Identity, bias=bp_sb[:, cft:cft + 1], scale=1.0)
                        nc.sync.dma_start(
                            out_d[wl, cft, :, nch * 512:(nch + 1) * 512],
                            outsb[:, cft * NTOK + nch * 512:cft * NTOK + (nch + 1) * 512])
            ph2.close()

    _split_sync_waits(nc, mybir, maxw=1)
    return nc


def _host_prepost(x, w_qkv, b_qkv):
    """Window partition, routing."""
    xw = x.reshape(T, B, WT, GT, WH, GH, WW, GW, C) \
          .transpose(0, 1, 2, 4, 6, 3, 5, 7, 8).reshape(T, B, W, S, C)
    xbar = xw.mean(axis=(0, 3))                      # [B, W, C]
    q_reg = xbar @ w_qkv[:, :C] + b_qkv[:C]
    k_reg = xbar @ w_qkv[:, C:2 * C] + b_qkv[C:2 * C]
    a_r = np.einsum('bwc,bvc->bwv', q_reg, k_reg)
    routing_idx = np.argsort(-a_r, axis=-1)[:, :, :TOPK]   # [B, W, TOPK]
    return xw, routing_idx


def _hi_lo16(a):
    hi = a.astype(np.float16)
    lo = (a - hi.astype(np.float32)).astype(np.float16)
    return hi, lo


def _hi_lo_bf(a):
    hi = a.astype(ml_dtypes.bfloat16)
    lo = (a - hi.astype(np.float32)).astype(ml_dtypes.bfloat16)
    return hi, lo


def kernel(x, w_qkv, b_qkv, w_proj, b_proj):
    x = np.ascontiguousarray(np.asarray(x, dtype=np.float32))
    w_qkv = np.asarray(w_qkv, dtype=np.float32)
    b_qkv = np.asarray(b_qkv, dtype=np.float32)
    w_proj = np.asarray(w_proj, dtype=np.float32)
    b_proj = np.asarray(b_proj, dtype=np.float32)

    xw, routing_idx = _host_prepost(x, w_qkv, b_qkv)

    key = (routing_idx.tobytes(), QKV_MODE, QE_MODE, GE_ENG)
    if key not in _prog_cache:
        _prog_cache.clear()
        _prog_cache[key] = _build_program(routing_idx)
    nc = _prog_cache[key]

    split_x = QKV_MODE in ("f16x1", "f16x2", "f16x3")
    NSPX = 2 if QKV_MODE in ("f16x2", "f16x3") else 1
    NSPW = 2 if QKV_MODE == "f16x3" else 1
    np_qkv = np.float16 if split_x else np.float32
    tscale = np.array([2.0 ** (t - 1) for t in range(T)], np.float32)

    # weights (shared across cores)
    wkv = w_qkv[:, C:].astype(np.float32)
    wq = w_qkv[:, :C].astype(np.float32)
    if NSPW == 2:
        wkv_hi, wkv_lo = _hi_lo16(wkv)
        wkv_arr = np.stack([wkv_hi, wkv_lo]).reshape(2, 2, 128, 512)
        wq_hi, wq_lo = _hi_lo16(wq)
        wq_arr = np.stack([wq_hi, wq_lo]).reshape(2, 2, 128, 256)
    else:
        wkv_arr = wkv.reshape(1, 2, 128, 512)
        wq_arr = wq.reshape(1, 2, 128, 256)

    bkv_arr = np.empty((T, 2, 512), dtype=ml_dtypes.bfloat16)
    bq_arr = np.empty((T, 2, 256), dtype=ml_dtypes.bfloat16)
    for t in range(T):
        hi, lo = _hi_lo_bf(tscale[t] * b_qkv[C:])
        bkv_arr[t, 0], bkv_arr[t, 1] = hi, lo
        hi, lo = _hi_lo_bf(tscale[t] * b_qkv[:C])
        bq_arr[t, 0], bq_arr[t, 1] = hi, lo

    wp = (SCALE * w_proj).reshape(2, 128, 256).astype(np.float16)
    bp = b_proj.reshape(2, 128, 1).astype(np.float32)

    in_maps = []
    for r in range(NCORES):
        b_of, wg = r // 4, r % 4
        xwc = xw[:, b_of, wg * 8:(wg + 1) * 8]              # [T, 8, S, C]
        xl32 = np.ascontiguousarray(
            xwc.transpose(1, 3, 0, 2))                      # [NW, C, T, S]
        xl32 = xl32 * tscale[None, None, :, None]
        xl32 = xl32.reshape(NW, 2, 128, NTOK)
        if split_x:
            xhi = xl32.astype(np.float16)
            xlo = (xl32 - xhi.astype(np.float32)).astype(np.float16)
            xl = np.stack([xhi, xlo], axis=1)[:, :NSPX]     # [NW, NSPX, 2, 128, NTOK]
        else:
            xl = xl32.reshape(NW, 1, 2, 128, NTOK)
        in_maps.append({
            "x_in": np.ascontiguousarray(xl.astype(np_qkv)),
            "wkv_in": wkv_arr.astype(np_qkv), "wq_in": wq_arr.astype(np_qkv),
            "bkv_in": bkv_arr, "bq_in": bq_arr,
            "wp_in": wp, "bp_in": bp,
        })

    from concourse.bass_utils import run_bass_kernel_spmd
    res = run_bass_kernel_spmd(nc, in_maps, core_ids=list(range(NCORES)))

    # assemble output
    yw = np.empty((T, B, W, S, C), dtype=np.float32)
    for r in range(NCORES):
        b_of, wg = r // 4, r % 4
        o = res.results[r]["out_d"]                          # [NW, 2, 128, NTOK]
        o = o.reshape(NW, 2, 128, T, S).transpose(0, 3, 4, 1, 2).reshape(NW, T, S, C)
        for wl in range(NW):
            yw[:, b_of, wg * 8 + wl] = o[wl]

    y = yw.reshape(T, B, WT, WH, WW, GT, GH, GW, C) \
          .transpose(0, 1, 2, 5, 3, 6, 4, 7, 8).reshape(T, B, Lt, Lh, Lw, C)
    return y
